# revision 15
# baseline (speedup 1.0000x reference)
"""GCN link-prediction kernel for 8 Trainium2 NeuronCores.

Strategy (target-sharded edges, replicated-by-AllGather node tables):
  - Nodes split into 8 contiguous shards. Each core computes its shard of
    g1 = dinv * (x @ W1) on PE, AllGather -> full table gtab1 in every
    core's HBM.
  - Train edges assigned to the core owning the TARGET node, grouped per
    128-target tile, padded to a fixed chunk count. Per 128-edge chunk:
    indirect-DMA gather of source rows, a DVE is_equal indicator matrix
    [edges x targets], and one PE matmul accumulating into PSUM.
    Self-loops (weight 2) are a per-tile extra chunk whose indicator is
    doubled.
  - Layer epilogue fuses dinv scaling, bias, relu, and the next layer's
    dense matmul (transposed via PE) so h1/h2 never round-trip to DRAM.
  - Edge head: z = h2 @ Wl1 table (64 f32), two gathers per 128-edge
    chunk, elementwise ops + free-dim reduction + sigmoid.

All float math runs on device in fp32; host only reorders/pads integer
edge indices and computes dinv (index-derived degree scaling).
"""
import sys
import os
import numpy as np

sys.path.insert(0, '/opt/trn_rl_repo')

N_CORES = 8
N = 50000
F_IN, H1, H2, H3 = 256, 256, 128, 64
SELF_LOOP_W = 2.0

NS = N // N_CORES            # 6250 nodes per shard
NT = (NS + 127) // 128       # 49 target tiles per core
NSP = NT * 128               # 6272 padded shard rows
HEAD_E = 400000
HE_CORE = HEAD_E // N_CORES  # 50000 head edges per core
NHC = (HE_CORE + 127) // 128  # 391 head chunks

_CACHE = {}


def _build_and_compile(Cts, variant='full'):
    """Build the SPMD Bass program. Cts[t] = data-chunks for target tile t."""
    import concourse.bass as bass
    import concourse.mybir as mybir
    import concourse.tile as tile
    from concourse import bacc

    dt = mybir.dt
    Cts = list(Cts)
    CHT = sum(Cts)        # data chunks per core (self-loop via direct DMA)
    bases = np.cumsum([0] + Cts)[:-1]

    nc = bacc.Bacc("TRN2", target_bir_lowering=False, debug=False,
                   num_devices=N_CORES)

    # ---- I/O ----
    xT = nc.dram_tensor("xT", [F_IN, NSP], dt.bfloat16, kind="ExternalInput")
    dinv_pm = nc.dram_tensor("dinv_pm", [128, NT], dt.float32, kind="ExternalInput")
    W1 = nc.dram_tensor("W1", [F_IN, H1], dt.bfloat16, kind="ExternalInput")
    W2 = nc.dram_tensor("W2", [H1, H2], dt.bfloat16, kind="ExternalInput")
    Wl1 = nc.dram_tensor("Wl1", [H2, H3], dt.bfloat16, kind="ExternalInput")
    b1t = nc.dram_tensor("b1t", [128, H1], dt.float32, kind="ExternalInput")
    b2t = nc.dram_tensor("b2t", [128, H2], dt.float32, kind="ExternalInput")
    bl1t = nc.dram_tensor("bl1t", [128, H3], dt.float32, kind="ExternalInput")
    wl2bc = nc.dram_tensor("wl2bc", [128, H3], dt.float32, kind="ExternalInput")
    bl2c = nc.dram_tensor("bl2c", [128, 1], dt.float32, kind="ExternalInput")
    esrc = nc.dram_tensor("esrc", [128, CHT], dt.int32, kind="ExternalInput")
    colloc = nc.dram_tensor("colloc", [128, CHT], dt.bfloat16, kind="ExternalInput")
    hsrc0 = nc.dram_tensor("hsrc0", [128, NHC], dt.int32, kind="ExternalInput")
    hsrc1 = nc.dram_tensor("hsrc1", [128, NHC], dt.int32, kind="ExternalInput")
    out_head = nc.dram_tensor("out_head", [128, NHC], dt.float32,
                              kind="ExternalOutput")

    from concourse.masks import make_identity

    with tile.TileContext(nc) as tc:
        with tc.tile_pool(name="const", bufs=1) as cpool, \
             tc.tile_pool(name="dram", bufs=1, space="DRAM") as dpool, \
             tc.tile_pool(name="gat", bufs=12) as gat_pool, \
             tc.tile_pool(name="ind", bufs=12) as ind_pool, \
             tc.tile_pool(name="work", bufs=6) as work, \
             tc.tile_pool(name="psA", bufs=3, space="PSUM") as psA, \
             tc.tile_pool(name="psT", bufs=2, space="PSUM") as psT:

            # ---- constants / index preload ----
            ident = cpool.tile([128, 128], dt.float32)
            make_identity(nc, ident[:])
            iota_i = cpool.tile([128, 128], dt.int32)
            nc.gpsimd.iota(iota_i[:], pattern=[[1, 128]], base=0,
                           channel_multiplier=0)
            iota_f = cpool.tile([128, 128], dt.bfloat16)
            nc.vector.tensor_copy(out=iota_f[:], in_=iota_i[:])
            ident2f = cpool.tile([128, 128], dt.float32)
            nc.vector.tensor_scalar_mul(ident2f[:], ident[:], float(SELF_LOOP_W))
            ident2 = cpool.tile([128, 128], dt.bfloat16)
            nc.vector.tensor_copy(out=ident2[:], in_=ident2f[:])

            W1s = cpool.tile([128, 2 * H1], dt.bfloat16)
            nc.sync.dma_start(out=W1s[:, :H1], in_=W1[0:128, :])
            nc.sync.dma_start(out=W1s[:, H1:], in_=W1[128:256, :])
            W2s = cpool.tile([128, 2 * H2], dt.bfloat16)
            nc.sync.dma_start(out=W2s[:, :H2], in_=W2[0:128, :])
            nc.sync.dma_start(out=W2s[:, H2:], in_=W2[128:256, :])
            Wl1s = cpool.tile([128, H3], dt.bfloat16)
            nc.sync.dma_start(out=Wl1s[:], in_=Wl1[:])
            b1s = cpool.tile([128, H1], dt.float32)
            nc.sync.dma_start(out=b1s[:], in_=b1t[:])
            b2s = cpool.tile([128, H2], dt.float32)
            nc.sync.dma_start(out=b2s[:], in_=b2t[:])
            bl1s = cpool.tile([128, H3], dt.float32)
            nc.sync.dma_start(out=bl1s[:], in_=bl1t[:])
            wl2s = cpool.tile([128, H3], dt.float32)
            nc.sync.dma_start(out=wl2s[:], in_=wl2bc[:])
            bl2s = cpool.tile([128, 1], dt.float32)
            nc.sync.dma_start(out=bl2s[:], in_=bl2c[:])
            dinv_s = cpool.tile([128, NT], dt.float32)
            nc.sync.dma_start(out=dinv_s[:], in_=dinv_pm[:])
            esrc_s = cpool.tile([128, CHT], dt.int32)
            nc.sync.dma_start(out=esrc_s[:], in_=esrc[:])
            colloc_s = cpool.tile([128, CHT], dt.bfloat16)
            nc.sync.dma_start(out=colloc_s[:], in_=colloc[:])
            h0_s = cpool.tile([128, NHC], dt.int32)
            nc.sync.dma_start(out=h0_s[:], in_=hsrc0[:])
            h1_s = cpool.tile([128, NHC], dt.int32)
            nc.sync.dma_start(out=h1_s[:], in_=hsrc1[:])

            # ---- DRAM internals ----
            g1_loc = dpool.tile([NS, H1], dt.bfloat16)
            gtab1 = dpool.tile([N, H1], dt.bfloat16, addr_space="Shared")
            g2_loc = dpool.tile([NS, H2], dt.bfloat16)
            gtab2 = dpool.tile([N, H2], dt.bfloat16, addr_space="Shared")
            z_loc = dpool.tile([NS, H3], dt.bfloat16)
            ztab = dpool.tile([N, H3], dt.bfloat16, addr_space="Shared")
            if variant == 'localtab':
                gtab1L = dpool.tile([N, H1], dt.bfloat16)
                gtab2L = dpool.tile([N, H2], dt.bfloat16)
                ztabL = dpool.tile([N, H3], dt.bfloat16)

            rg = [list(range(N_CORES))]

            # ================= Phase A: g1 shard =================
            for t in range(NT):
                rows = min(128, NS - t * 128)
                xa = work.tile([128, 128], dt.bfloat16, tag="xa")
                xb = work.tile([128, 128], dt.bfloat16, tag="xb")
                nc.sync.dma_start(out=xa[:], in_=xT[0:128, t * 128:(t + 1) * 128])
                nc.sync.dma_start(out=xb[:], in_=xT[128:256, t * 128:(t + 1) * 128])
                ps = psA.tile([128, H1], dt.float32, tag="psagg")
                nc.tensor.matmul(out=ps[:], lhsT=xa[:], rhs=W1s[:, :H1],
                                 start=True, stop=False)
                nc.tensor.matmul(out=ps[:], lhsT=xb[:], rhs=W1s[:, H1:],
                                 start=False, stop=True)
                g1v = work.tile([128, H1], dt.bfloat16, tag="g1v")
                nc.vector.tensor_mul(
                    out=g1v[:], in0=ps[:],
                    in1=dinv_s[:, t:t + 1].to_broadcast([128, H1]))
                nc.sync.dma_start(out=g1_loc[t * 128: t * 128 + rows, :],
                                  in_=g1v[:rows, :])
            if variant != 'nocoll':
                nc.gpsimd.collective_compute(
                    "AllGather", mybir.AluOpType.bypass, replica_groups=rg,
                    ins=[g1_loc.opt()], outs=[gtab1.opt()])
            if variant == 'localtab':
                nc.sync.dma_start(out=gtab1L[:, :], in_=gtab1[:, :])
                gtab1 = gtab1L

            # ============ Layer helpers ============
            def agg_layer(gtab, F, gloc):
                """Yields per-tile psum [128, F]: self-loop (x2, direct DMA
                load from the core-local shard) + Cts[t] gathered chunks."""
                for t in range(NT):
                    rows = min(128, NS - t * 128)
                    # self chunk: contiguous rows of own shard, indicator 2*I
                    gs = gat_pool.tile([128, F], dt.bfloat16, tag="gath")
                    nc.sync.dma_start(
                        out=gs[:rows, :],
                        in_=gloc[t * 128: t * 128 + rows, :])
                    if variant == 'dmaonly':
                        ps = None
                    else:
                        ps = psA.tile([128, F], dt.float32, tag="psagg")
                        nc.tensor.matmul(out=ps[:], lhsT=ident2[:], rhs=gs[:],
                                         start=True,
                                         stop=(variant in ('nomm', 'noagg')))
                    for c in range(0 if variant == 'noagg' else Cts[t]):
                        j = int(bases[t]) + c
                        g = gat_pool.tile([128, F], dt.bfloat16, tag="gath")
                        nc.gpsimd.indirect_dma_start(
                            out=g[:], out_offset=None, in_=gtab[:],
                            in_offset=bass.IndirectOffsetOnAxis(
                                ap=esrc_s[:, j:j + 1], axis=0))
                        if variant == 'dmaonly':
                            continue
                        if variant in ('noind', 'nomm'):
                            ind = iota_f
                        else:
                            ind = ind_pool.tile([128, 128], dt.bfloat16,
                                                tag="ind")
                            nc.vector.tensor_tensor(
                                out=ind[:],
                                in0=colloc_s[:, j:j + 1].to_broadcast([128, 128]),
                                in1=iota_f[:], op=mybir.AluOpType.is_equal)
                        if variant != 'nomm':
                            nc.tensor.matmul(out=ps[:], lhsT=ind[:], rhs=g[:],
                                             start=False,
                                             stop=(c == Cts[t] - 1))
                    yield t, ps

            # ============ Layer 1 + fused g2 ============
            for t, ps in agg_layer(gtab1, H1, g1_loc):
                rows = min(128, NS - t * 128)
                if variant == 'dmaonly':
                    g2v = work.tile([128, H2], dt.bfloat16, tag="g2v")
                    nc.sync.dma_start(out=g2_loc[t * 128: t * 128 + rows, :],
                                      in_=g2v[:rows, :])
                    continue
                dv = dinv_s[:, t:t + 1]
                h1v = work.tile([128, H1], dt.float32, tag="h1v")
                nc.vector.tensor_mul(out=h1v[:], in0=ps[:],
                                     in1=dv.to_broadcast([128, H1]))
                nc.vector.tensor_add(out=h1v[:], in0=h1v[:], in1=b1s[:])
                nc.scalar.activation(out=h1v[:], in_=h1v[:],
                                     func=mybir.ActivationFunctionType.Relu)
                nc.vector.tensor_mul(out=h1v[:], in0=h1v[:],
                                     in1=dv.to_broadcast([128, H1]))
                # transpose h1d -> [feat, rows], then g2 = h1d @ W2
                g2ps = psA.tile([128, H2], dt.float32, tag="pssm")
                tpss = []
                for fb in range(2):
                    tp = psT.tile([128, 128], dt.float32, tag="tp")
                    nc.tensor.transpose(out=tp[:],
                                        in_=h1v[:, fb * 128:(fb + 1) * 128],
                                        identity=ident[:])
                    tps = work.tile([128, 128], dt.bfloat16, tag=f"tps{fb}")
                    nc.vector.tensor_copy(out=tps[:], in_=tp[:])
                    tpss.append(tps)
                for fb in range(2):
                    nc.tensor.matmul(out=g2ps[:], lhsT=tpss[fb][:],
                                     rhs=W2s[:, fb * H2:(fb + 1) * H2],
                                     start=(fb == 0), stop=(fb == 1))
                g2v = work.tile([128, H2], dt.bfloat16, tag="g2v")
                nc.vector.tensor_copy(out=g2v[:], in_=g2ps[:])
                nc.sync.dma_start(out=g2_loc[t * 128: t * 128 + rows, :],
                                  in_=g2v[:rows, :])
            if variant != 'nocoll':
                nc.gpsimd.collective_compute(
                    "AllGather", mybir.AluOpType.bypass, replica_groups=rg,
                    ins=[g2_loc.opt()], outs=[gtab2.opt()])
            if variant == 'localtab':
                nc.sync.dma_start(out=gtab2L[:, :], in_=gtab2[:, :])
                gtab2 = gtab2L

            # ============ Layer 2 + fused z ============
            for t, ps in agg_layer(gtab2, H2, g2_loc):
                rows = min(128, NS - t * 128)
                if variant == 'dmaonly':
                    zv = work.tile([128, H3], dt.bfloat16, tag="zv")
                    nc.sync.dma_start(out=z_loc[t * 128: t * 128 + rows, :],
                                      in_=zv[:rows, :])
                    continue
                dv = dinv_s[:, t:t + 1]
                h2v = work.tile([128, H2], dt.float32, tag="h2v")
                nc.vector.tensor_mul(out=h2v[:], in0=ps[:],
                                     in1=dv.to_broadcast([128, H2]))
                nc.vector.tensor_add(out=h2v[:], in0=h2v[:], in1=b2s[:])
                tp = psT.tile([128, 128], dt.float32, tag="tp")
                nc.tensor.transpose(out=tp[:], in_=h2v[:], identity=ident[:])
                tps = work.tile([128, 128], dt.bfloat16, tag="tps")
                nc.vector.tensor_copy(out=tps[:], in_=tp[:])
                zps = psA.tile([128, H3], dt.float32, tag="pssm")
                nc.tensor.matmul(out=zps[:], lhsT=tps[:], rhs=Wl1s[:],
                                 start=True, stop=True)
                zv = work.tile([128, H3], dt.bfloat16, tag="zv")
                nc.vector.tensor_copy(out=zv[:], in_=zps[:])
                nc.sync.dma_start(out=z_loc[t * 128: t * 128 + rows, :],
                                  in_=zv[:rows, :])
            if variant != 'nocoll':
                nc.gpsimd.collective_compute(
                    "AllGather", mybir.AluOpType.bypass, replica_groups=rg,
                    ins=[z_loc.opt()], outs=[ztab.opt()])
            if variant == 'localtab':
                nc.sync.dma_start(out=ztabL[:, :], in_=ztab[:, :])
                ztab = ztabL

            # ============ Edge head ============
            out_sb = cpool.tile([128, NHC], dt.float32)
            if variant in ('nohead', 'dmaonly'):
                nc.gpsimd.memset(out_sb[:], 0)
            for c in range(0 if variant == 'nohead' else NHC):
                r0 = gat_pool.tile([128, H3], dt.bfloat16, tag="hg0")
                nc.gpsimd.indirect_dma_start(
                    out=r0[:], out_offset=None, in_=ztab[:],
                    in_offset=bass.IndirectOffsetOnAxis(
                        ap=h0_s[:, c:c + 1], axis=0))
                r1 = gat_pool.tile([128, H3], dt.bfloat16, tag="hg1")
                nc.gpsimd.indirect_dma_start(
                    out=r1[:], out_offset=None, in_=ztab[:],
                    in_offset=bass.IndirectOffsetOnAxis(
                        ap=h1_s[:, c:c + 1], axis=0))
                if variant == 'dmaonly':
                    continue
                e1 = work.tile([128, H3], dt.float32, tag="e1")
                nc.vector.tensor_add(out=e1[:], in0=r0[:], in1=r1[:])
                nc.vector.tensor_add(out=e1[:], in0=e1[:], in1=bl1s[:])
                nc.scalar.activation(out=e1[:], in_=e1[:],
                                     func=mybir.ActivationFunctionType.Relu)
                nc.vector.tensor_mul(out=e1[:], in0=e1[:], in1=wl2s[:])
                sc = work.tile([128, 1], dt.float32, tag="sc")
                nc.vector.reduce_sum(out=sc[:], in_=e1[:],
                                     axis=mybir.AxisListType.X)
                nc.scalar.activation(out=out_sb[:, c:c + 1], in_=sc[:],
                                     func=mybir.ActivationFunctionType.Sigmoid,
                                     bias=bl2s[:])
            nc.sync.dma_start(out=out_head[:], in_=out_sb[:])

    nc.compile()
    return nc


def _prep_inputs(x, train_edge_index, pos_edge_index, neg_edge_index,
                 W1, b1, W2, b2, Wl1, bl1, Wl2, bl2):
    """Host-side sharding / index layout. Returns (in_maps, C)."""
    x = np.asarray(x, np.float32)
    ei = np.asarray(train_edge_index)
    row, col = ei[0].astype(np.int64), ei[1].astype(np.int64)
    deg = np.bincount(col, minlength=N).astype(np.float32) + SELF_LOOP_W
    dinv = (1.0 / np.sqrt(deg)).astype(np.float32)

    import ml_dtypes
    bf16 = ml_dtypes.bfloat16
    W1 = np.asarray(W1, np.float32).astype(bf16)
    W2 = np.asarray(W2, np.float32).astype(bf16)
    Wl1 = np.asarray(Wl1, np.float32).astype(bf16)
    b1 = np.asarray(b1, np.float32)
    b2 = np.asarray(b2, np.float32)
    bl1 = np.asarray(bl1, np.float32)
    Wl2 = np.asarray(Wl2, np.float32).reshape(-1)
    bl2 = np.asarray(bl2, np.float32).reshape(-1)

    # --- per-(core,tile) edge grouping ---
    core_of = col // NS
    tile_of = (col % NS) // 128
    # chunk requirement per (core, tile)
    counts = np.zeros((N_CORES, NT), np.int64)
    np.add.at(counts, (core_of, tile_of), 1)
    Cts = [int(np.ceil(counts[:, t].max() / 128.0)) for t in range(NT)]
    bases = np.cumsum([0] + Cts)[:-1]
    CHT = int(sum(Cts))

    order = np.lexsort((tile_of, core_of))
    row_s, col_s = row[order], col[order]
    core_s, tile_s = core_of[order], tile_of[order]
    # boundaries per (core,tile)
    grp = core_s * NT + tile_s
    starts = np.searchsorted(grp, np.arange(N_CORES * NT))
    ends = np.searchsorted(grp, np.arange(N_CORES * NT), side='right')

    tei = np.concatenate([np.asarray(pos_edge_index),
                          np.asarray(neg_edge_index)], axis=-1)
    t0_all, t1_all = tei[0].astype(np.int64), tei[1].astype(np.int64)

    in_maps = []
    for k in range(N_CORES):
        lo = k * NS
        esrc = np.zeros((128, CHT), np.int32)
        colloc = np.full((128, CHT), -1.0, np.float32)  # cast to bf16 at pack
        for t in range(NT):
            Ct = Cts[t]
            base = int(bases[t])
            s, e = starts[k * NT + t], ends[k * NT + t]
            ne = e - s
            assert ne <= Ct * 128, "chunk overflow"
            srcs = row_s[s:e]
            locs = (col_s[s:e] - lo - t * 128).astype(np.float32)
            full = np.zeros(Ct * 128, np.int32)
            fullc = np.full(Ct * 128, -1.0, np.float32)
            full[:ne] = srcs
            fullc[:ne] = locs
            esrc[:, base: base + Ct] = full.reshape(Ct, 128).T
            colloc[:, base: base + Ct] = fullc.reshape(Ct, 128).T  # cast below

        # head edges
        h0 = np.zeros(NHC * 128, np.int32)
        h1 = np.zeros(NHC * 128, np.int32)
        h0[:HE_CORE] = t0_all[k * HE_CORE:(k + 1) * HE_CORE]
        h1[:HE_CORE] = t1_all[k * HE_CORE:(k + 1) * HE_CORE]
        hsrc0 = h0.reshape(NHC, 128).T.copy()
        hsrc1 = h1.reshape(NHC, 128).T.copy()

        # node shard data
        xs = np.zeros((NSP, F_IN), np.float32)
        xs[:NS] = x[lo:lo + NS]
        xT = np.ascontiguousarray(xs.T).astype(bf16)
        dpm = np.zeros((128, NT), np.float32)
        dsh = np.zeros(NSP, np.float32)
        dsh[:NS] = dinv[lo:lo + NS]
        dpm[:, :] = dsh.reshape(NT, 128).T

        in_maps.append({
            "xT": xT, "dinv_pm": dpm,
            "W1": W1, "W2": W2, "Wl1": Wl1,
            "b1t": np.tile(b1[None, :], (128, 1)),
            "b2t": np.tile(b2[None, :], (128, 1)),
            "bl1t": np.tile(bl1[None, :], (128, 1)),
            "wl2bc": np.tile(Wl2[None, :], (128, 1)),
            "bl2c": np.full((128, 1), bl2[0], np.float32),
            "esrc": esrc, "colloc": colloc.astype(bf16),
            "hsrc0": hsrc0, "hsrc1": hsrc1,
        })
    return in_maps, tuple(Cts)


def _get_runner(C, in_maps):
    import jax
    from concourse import bass2jax, mybir as mb
    from jax.sharding import Mesh, PartitionSpec
    from jax.experimental.shard_map import shard_map

    key = ("runner", C)
    if key in _CACHE:
        return _CACHE[key]

    nc = _CACHE.get(("nc", C))
    if nc is None:
        nc = _build_and_compile(C)
        _CACHE[("nc", C)] = nc

    bass2jax.install_neuronx_cc_hook()
    partition_name = nc.partition_id_tensor.name if nc.partition_id_tensor else None
    in_names, out_names, out_avals, zero_outs = [], [], [], []
    for a in nc.m.functions[0].allocations:
        if not isinstance(a, mb.MemoryLocationSet):
            continue
        name = a.memorylocations[0].name
        if a.kind == "ExternalInput":
            if name != partition_name:
                in_names.append(name)
        elif a.kind == "ExternalOutput":
            out_names.append(name)
            shape = tuple(a.tensor_shape)
            dtype = mb.dt.np(a.dtype)
            out_avals.append(jax.core.ShapedArray(shape, dtype))
            zero_outs.append(np.zeros(shape, dtype))
    n_params = len(in_names)
    all_in_names = in_names + out_names + ([partition_name] if partition_name else [])

    def _body(*args):
        operands = list(args)
        if partition_name is not None:
            operands.append(bass2jax.partition_id_tensor())
        outs = bass2jax._bass_exec_p.bind(
            *operands, out_avals=tuple(out_avals), in_names=tuple(all_in_names),
            out_names=tuple(out_names), lowering_input_output_aliases=(),
            sim_require_finite=True, sim_require_nnan=True, nc=nc)
        return tuple(outs)

    devices = jax.devices()[:N_CORES]
    mesh = Mesh(np.asarray(devices), ("core",))
    in_specs = (PartitionSpec("core"),) * (n_params + len(out_names))
    out_specs = (PartitionSpec("core"),) * len(out_names)
    sharded = jax.jit(shard_map(_body, mesh=mesh, in_specs=in_specs,
                                out_specs=out_specs, check_rep=False),
                      keep_unused=True)

    def run(maps):
        concat_in = [np.concatenate([np.asarray(maps[c][nm])
                                     for c in range(N_CORES)], axis=0)
                     for nm in in_names]
        concat_zero = [np.concatenate([z] * N_CORES, axis=0) for z in zero_outs]
        outs = sharded(*concat_in, *concat_zero)
        jax.block_until_ready(outs)
        return {nm: np.asarray(outs[i]) for i, nm in enumerate(out_names)}

    _CACHE[key] = run
    return run


def kernel(**inputs) -> np.ndarray:
    in_maps, C = _prep_inputs(**inputs)
    run = _get_runner(C, in_maps)
    outs = run(in_maps)
    oh = outs["out_head"].reshape(N_CORES, 128, NHC)
    res = np.empty(HEAD_E, np.float32)
    for k in range(N_CORES):
        flat = oh[k].T.reshape(-1)  # [NHC*128] in edge order
        res[k * HE_CORE:(k + 1) * HE_CORE] = flat[:HE_CORE]
    return res



# revision 17
# speedup vs baseline: 1.3418x; 1.3418x over previous
"""GCN link-prediction kernel for 8 Trainium2 NeuronCores.

Strategy (target-sharded edges, replicated-by-AllGather node tables):
  - Nodes split into 8 contiguous shards. Each core computes its shard of
    g1 = dinv * (x @ W1) on PE, AllGather -> full table gtab1 in every
    core's HBM.
  - Train edges assigned to the core owning the TARGET node, grouped per
    128-target tile, padded to a fixed chunk count. Per 128-edge chunk:
    indirect-DMA gather of source rows, a DVE is_equal indicator matrix
    [edges x targets], and one PE matmul accumulating into PSUM.
    Self-loops (weight 2) are a per-tile extra chunk whose indicator is
    doubled.
  - Layer epilogue fuses dinv scaling, bias, relu, and the next layer's
    dense matmul (transposed via PE) so h1/h2 never round-trip to DRAM.
  - Edge head: z = h2 @ Wl1 table (64 f32), two gathers per 128-edge
    chunk, elementwise ops + free-dim reduction + sigmoid.

All float math runs on device in fp32; host only reorders/pads integer
edge indices and computes dinv (index-derived degree scaling).
"""
import sys
import os
import numpy as np

sys.path.insert(0, '/opt/trn_rl_repo')

N_CORES = 8
N = 50000
F_IN, H1, H2, H3 = 256, 256, 128, 64
SELF_LOOP_W = 2.0

NS = N // N_CORES            # 6250 nodes per shard
NT = (NS + 127) // 128       # 49 target tiles per core
NSP = NT * 128               # 6272 padded shard rows
HEAD_E = 400000
HE_CORE = HEAD_E // N_CORES  # 50000 head edges per core
NHC = (HE_CORE + 127) // 128  # 391 head chunks

_CACHE = {}


def _build_and_compile(Cts, variant='full'):
    """Build the SPMD Bass program. Cts[t] = data-chunks for target tile t."""
    import concourse.bass as bass
    import concourse.mybir as mybir
    import concourse.tile as tile
    from concourse import bacc

    dt = mybir.dt
    Cts = list(Cts)
    CHT = sum(Cts)        # data chunks per core (self-loop via direct DMA)
    bases = np.cumsum([0] + Cts)[:-1]

    nc = bacc.Bacc("TRN2", target_bir_lowering=False, debug=False,
                   num_devices=N_CORES)

    # ---- I/O ----
    xT = nc.dram_tensor("xT", [F_IN, NSP], dt.bfloat16, kind="ExternalInput")
    dinv_pm = nc.dram_tensor("dinv_pm", [128, NT], dt.float32, kind="ExternalInput")
    W1 = nc.dram_tensor("W1", [F_IN, H1], dt.bfloat16, kind="ExternalInput")
    W2 = nc.dram_tensor("W2", [H1, H2], dt.bfloat16, kind="ExternalInput")
    Wl1 = nc.dram_tensor("Wl1", [H2, H3], dt.bfloat16, kind="ExternalInput")
    b1t = nc.dram_tensor("b1t", [128, H1], dt.float32, kind="ExternalInput")
    b2t = nc.dram_tensor("b2t", [128, H2], dt.float32, kind="ExternalInput")
    bl1t = nc.dram_tensor("bl1t", [128, H3], dt.float32, kind="ExternalInput")
    wl2bc = nc.dram_tensor("wl2bc", [128, H3], dt.float32, kind="ExternalInput")
    bl2c = nc.dram_tensor("bl2c", [128, 1], dt.float32, kind="ExternalInput")
    esrc = nc.dram_tensor("esrc", [128, CHT], dt.int32, kind="ExternalInput")
    colloc = nc.dram_tensor("colloc", [128, CHT], dt.bfloat16, kind="ExternalInput")
    hsrc0 = nc.dram_tensor("hsrc0", [128, NHC], dt.int32, kind="ExternalInput")
    hsrc1 = nc.dram_tensor("hsrc1", [128, NHC], dt.int32, kind="ExternalInput")
    out_head = nc.dram_tensor("out_head", [128, NHC], dt.float32,
                              kind="ExternalOutput")

    from concourse.masks import make_identity

    with tile.TileContext(nc) as tc:
        with tc.tile_pool(name="const", bufs=1) as cpool, \
             tc.tile_pool(name="dram", bufs=1, space="DRAM") as dpool, \
             tc.tile_pool(name="gat", bufs=12) as gat_pool, \
             tc.tile_pool(name="ind", bufs=12) as ind_pool, \
             tc.tile_pool(name="work", bufs=6) as work, \
             tc.tile_pool(name="psA", bufs=3, space="PSUM") as psA, \
             tc.tile_pool(name="psT", bufs=2, space="PSUM") as psT:

            # ---- constants / index preload ----
            ident = cpool.tile([128, 128], dt.float32)
            make_identity(nc, ident[:])
            iota_i = cpool.tile([128, 128], dt.int32)
            nc.gpsimd.iota(iota_i[:], pattern=[[1, 128]], base=0,
                           channel_multiplier=0)
            iota_f = cpool.tile([128, 128], dt.bfloat16)
            nc.vector.tensor_copy(out=iota_f[:], in_=iota_i[:])
            ident2f = cpool.tile([128, 128], dt.float32)
            nc.vector.tensor_scalar_mul(ident2f[:], ident[:], float(SELF_LOOP_W))
            ident2 = cpool.tile([128, 128], dt.bfloat16)
            nc.vector.tensor_copy(out=ident2[:], in_=ident2f[:])

            W1s = cpool.tile([128, 2 * H1], dt.bfloat16)
            nc.sync.dma_start(out=W1s[:, :H1], in_=W1[0:128, :])
            nc.sync.dma_start(out=W1s[:, H1:], in_=W1[128:256, :])
            W2s = cpool.tile([128, 2 * H2], dt.bfloat16)
            nc.sync.dma_start(out=W2s[:, :H2], in_=W2[0:128, :])
            nc.sync.dma_start(out=W2s[:, H2:], in_=W2[128:256, :])
            Wl1s = cpool.tile([128, H3], dt.bfloat16)
            nc.sync.dma_start(out=Wl1s[:], in_=Wl1[:])
            b1s = cpool.tile([128, H1], dt.float32)
            nc.sync.dma_start(out=b1s[:], in_=b1t[:])
            b2s = cpool.tile([128, H2], dt.float32)
            nc.sync.dma_start(out=b2s[:], in_=b2t[:])
            bl1s = cpool.tile([128, H3], dt.float32)
            nc.sync.dma_start(out=bl1s[:], in_=bl1t[:])
            wl2s = cpool.tile([128, H3], dt.float32)
            nc.sync.dma_start(out=wl2s[:], in_=wl2bc[:])
            bl2s = cpool.tile([128, 1], dt.float32)
            nc.sync.dma_start(out=bl2s[:], in_=bl2c[:])
            dinv_s = cpool.tile([128, NT], dt.float32)
            nc.sync.dma_start(out=dinv_s[:], in_=dinv_pm[:])
            esrc_s = cpool.tile([128, CHT], dt.int32)
            nc.sync.dma_start(out=esrc_s[:], in_=esrc[:])
            colloc_s = cpool.tile([128, CHT], dt.bfloat16)
            nc.sync.dma_start(out=colloc_s[:], in_=colloc[:])
            h0_s = cpool.tile([128, NHC], dt.int32)
            nc.sync.dma_start(out=h0_s[:], in_=hsrc0[:])
            h1_s = cpool.tile([128, NHC], dt.int32)
            nc.sync.dma_start(out=h1_s[:], in_=hsrc1[:])

            # ---- DRAM internals ----
            NP = N_CORES * NSP
            g1_loc = dpool.tile([NSP, H1], dt.bfloat16)
            gtab1 = dpool.tile([NP, H1], dt.bfloat16, addr_space="Shared")
            g2_loc = dpool.tile([NSP, H2], dt.bfloat16)
            gtab2 = dpool.tile([NP, H2], dt.bfloat16, addr_space="Shared")
            z_loc = dpool.tile([NSP, H3], dt.bfloat16)
            ztab = dpool.tile([NP, H3], dt.bfloat16, addr_space="Shared")
            selfb1 = cpool.tile([128, NT * H1], dt.bfloat16)
            selfb2 = cpool.tile([128, NT * H2], dt.bfloat16)
            g2stage = cpool.tile([128, NT * H2], dt.bfloat16)
            zstage = cpool.tile([128, NT * H3], dt.bfloat16)
            if variant == 'localtab':
                gtab1L = dpool.tile([N, H1], dt.bfloat16)
                gtab2L = dpool.tile([N, H2], dt.bfloat16)
                ztabL = dpool.tile([N, H3], dt.bfloat16)

            rg = [list(range(N_CORES))]

            # ================= Phase A: g1 shard =================
            for t in range(NT):
                rows = min(128, NS - t * 128)
                xa = work.tile([128, 128], dt.bfloat16, tag="xa")
                xb = work.tile([128, 128], dt.bfloat16, tag="xb")
                nc.sync.dma_start(out=xa[:], in_=xT[0:128, t * 128:(t + 1) * 128])
                nc.sync.dma_start(out=xb[:], in_=xT[128:256, t * 128:(t + 1) * 128])
                ps = psA.tile([128, H1], dt.float32, tag="psagg")
                nc.tensor.matmul(out=ps[:], lhsT=xa[:], rhs=W1s[:, :H1],
                                 start=True, stop=False)
                nc.tensor.matmul(out=ps[:], lhsT=xb[:], rhs=W1s[:, H1:],
                                 start=False, stop=True)
                g1v = work.tile([128, H1], dt.bfloat16, tag="g1v")
                nc.vector.tensor_mul(
                    out=g1v[:], in0=ps[:],
                    in1=dinv_s[:, t:t + 1].to_broadcast([128, H1]))
                nc.sync.dma_start(out=g1_loc[t * 128: (t + 1) * 128, :],
                                  in_=g1v[:, :])
            if variant != 'nocoll':
                nc.gpsimd.collective_compute(
                    "AllGather", mybir.AluOpType.bypass, replica_groups=rg,
                    ins=[g1_loc.opt()], outs=[gtab1.opt()])
            if variant == 'localtab':
                nc.sync.dma_start(out=gtab1L[:, :], in_=gtab1[:, :])
                gtab1 = gtab1L

            # ============ Layer helpers ============
            def agg_layer(gtab, F, selfb):
                """Yields per-tile psum [128, F]: self-loop (x2, from the
                SBUF-resident self table) + Cts[t] gathered chunks."""
                for t in range(NT):
                    if variant == 'dmaonly':
                        ps = None
                    else:
                        ps = psA.tile([128, F], dt.float32, tag="psagg")
                        nc.tensor.matmul(out=ps[:], lhsT=ident2[:],
                                         rhs=selfb[:, t * F:(t + 1) * F],
                                         start=True,
                                         stop=(variant in ('nomm', 'noagg')))
                    for c in range(0 if variant == 'noagg' else Cts[t]):
                        j = int(bases[t]) + c
                        g = gat_pool.tile([128, F], dt.bfloat16, tag="gath")
                        nc.gpsimd.indirect_dma_start(
                            out=g[:], out_offset=None, in_=gtab[:],
                            in_offset=bass.IndirectOffsetOnAxis(
                                ap=esrc_s[:, j:j + 1], axis=0))
                        if variant == 'dmaonly':
                            continue
                        if variant in ('noind', 'nomm'):
                            ind = iota_f
                        else:
                            ind = ind_pool.tile([128, 128], dt.bfloat16,
                                                tag="ind")
                            nc.vector.tensor_tensor(
                                out=ind[:],
                                in0=colloc_s[:, j:j + 1].to_broadcast([128, 128]),
                                in1=iota_f[:], op=mybir.AluOpType.is_equal)
                        if variant != 'nomm':
                            nc.tensor.matmul(out=ps[:], lhsT=ind[:], rhs=g[:],
                                             start=False,
                                             stop=(c == Cts[t] - 1))
                    yield t, ps

            # ============ Layer 1 + fused g2 ============
            nc.sync.dma_start(
                out=selfb1[:].rearrange("p (t f) -> p t f", t=NT),
                in_=g1_loc[:].rearrange("(t p) f -> p t f", p=128))
            for t, ps in agg_layer(gtab1, H1, selfb1):
                rows = min(128, NS - t * 128)
                if variant == 'dmaonly':
                    g2v = work.tile([128, H2], dt.bfloat16, tag="g2v")
                    nc.sync.dma_start(out=g2_loc[t * 128: t * 128 + rows, :],
                                      in_=g2v[:rows, :])
                    continue
                dv = dinv_s[:, t:t + 1]
                h1v = work.tile([128, H1], dt.float32, tag="h1v")
                nc.vector.tensor_mul(out=h1v[:], in0=ps[:],
                                     in1=dv.to_broadcast([128, H1]))
                nc.vector.tensor_add(out=h1v[:], in0=h1v[:], in1=b1s[:])
                nc.scalar.activation(out=h1v[:], in_=h1v[:],
                                     func=mybir.ActivationFunctionType.Relu)
                nc.vector.tensor_mul(out=h1v[:], in0=h1v[:],
                                     in1=dv.to_broadcast([128, H1]))
                # transpose h1d -> [feat, rows], then g2 = h1d @ W2
                g2ps = psA.tile([128, H2], dt.float32, tag="pssm")
                tpss = []
                for fb in range(2):
                    tp = psT.tile([128, 128], dt.float32, tag="tp")
                    nc.tensor.transpose(out=tp[:],
                                        in_=h1v[:, fb * 128:(fb + 1) * 128],
                                        identity=ident[:])
                    tps = work.tile([128, 128], dt.bfloat16, tag=f"tps{fb}")
                    nc.vector.tensor_copy(out=tps[:], in_=tp[:])
                    tpss.append(tps)
                for fb in range(2):
                    nc.tensor.matmul(out=g2ps[:], lhsT=tpss[fb][:],
                                     rhs=W2s[:, fb * H2:(fb + 1) * H2],
                                     start=(fb == 0), stop=(fb == 1))
                nc.vector.tensor_copy(out=g2stage[:, t * H2:(t + 1) * H2],
                                       in_=g2ps[:])
            nc.sync.dma_start(
                out=g2_loc[:].rearrange("(t p) f -> p t f", p=128),
                in_=g2stage[:].rearrange("p (t f) -> p t f", t=NT))
            if variant != 'nocoll':
                nc.gpsimd.collective_compute(
                    "AllGather", mybir.AluOpType.bypass, replica_groups=rg,
                    ins=[g2_loc.opt()], outs=[gtab2.opt()])
            if variant == 'localtab':
                nc.sync.dma_start(out=gtab2L[:, :], in_=gtab2[:, :])
                gtab2 = gtab2L

            # ============ Layer 2 + fused z ============
            nc.sync.dma_start(
                out=selfb2[:].rearrange("p (t f) -> p t f", t=NT),
                in_=g2_loc[:].rearrange("(t p) f -> p t f", p=128))
            for t, ps in agg_layer(gtab2, H2, selfb2):
                rows = min(128, NS - t * 128)
                if variant == 'dmaonly':
                    zv = work.tile([128, H3], dt.bfloat16, tag="zv")
                    nc.sync.dma_start(out=z_loc[t * 128: t * 128 + rows, :],
                                      in_=zv[:rows, :])
                    continue
                dv = dinv_s[:, t:t + 1]
                h2v = work.tile([128, H2], dt.float32, tag="h2v")
                nc.vector.tensor_mul(out=h2v[:], in0=ps[:],
                                     in1=dv.to_broadcast([128, H2]))
                nc.vector.tensor_add(out=h2v[:], in0=h2v[:], in1=b2s[:])
                tp = psT.tile([128, 128], dt.float32, tag="tp")
                nc.tensor.transpose(out=tp[:], in_=h2v[:], identity=ident[:])
                tps = work.tile([128, 128], dt.bfloat16, tag="tps")
                nc.vector.tensor_copy(out=tps[:], in_=tp[:])
                zps = psA.tile([128, H3], dt.float32, tag="pssm")
                nc.tensor.matmul(out=zps[:], lhsT=tps[:], rhs=Wl1s[:],
                                 start=True, stop=True)
                nc.vector.tensor_copy(out=zstage[:, t * H3:(t + 1) * H3],
                                       in_=zps[:])
            nc.sync.dma_start(
                out=z_loc[:].rearrange("(t p) f -> p t f", p=128),
                in_=zstage[:].rearrange("p (t f) -> p t f", t=NT))
            if variant != 'nocoll':
                nc.gpsimd.collective_compute(
                    "AllGather", mybir.AluOpType.bypass, replica_groups=rg,
                    ins=[z_loc.opt()], outs=[ztab.opt()])
            if variant == 'localtab':
                nc.sync.dma_start(out=ztabL[:, :], in_=ztab[:, :])
                ztab = ztabL

            # ============ Edge head ============
            out_sb = cpool.tile([128, NHC], dt.float32)
            if variant in ('nohead', 'dmaonly'):
                nc.gpsimd.memset(out_sb[:], 0)
            for c in range(0 if variant == 'nohead' else NHC):
                r0 = gat_pool.tile([128, H3], dt.bfloat16, tag="hg0")
                nc.gpsimd.indirect_dma_start(
                    out=r0[:], out_offset=None, in_=ztab[:],
                    in_offset=bass.IndirectOffsetOnAxis(
                        ap=h0_s[:, c:c + 1], axis=0))
                r1 = gat_pool.tile([128, H3], dt.bfloat16, tag="hg1")
                nc.gpsimd.indirect_dma_start(
                    out=r1[:], out_offset=None, in_=ztab[:],
                    in_offset=bass.IndirectOffsetOnAxis(
                        ap=h1_s[:, c:c + 1], axis=0))
                if variant == 'dmaonly':
                    continue
                e1 = work.tile([128, H3], dt.float32, tag="e1")
                nc.vector.tensor_add(out=e1[:], in0=r0[:], in1=r1[:])
                nc.vector.tensor_add(out=e1[:], in0=e1[:], in1=bl1s[:])
                nc.scalar.activation(out=e1[:], in_=e1[:],
                                     func=mybir.ActivationFunctionType.Relu)
                nc.vector.tensor_mul(out=e1[:], in0=e1[:], in1=wl2s[:])
                sc = work.tile([128, 1], dt.float32, tag="sc")
                nc.vector.reduce_sum(out=sc[:], in_=e1[:],
                                     axis=mybir.AxisListType.X)
                nc.scalar.activation(out=out_sb[:, c:c + 1], in_=sc[:],
                                     func=mybir.ActivationFunctionType.Sigmoid,
                                     bias=bl2s[:])
            nc.sync.dma_start(out=out_head[:], in_=out_sb[:])

    nc.compile()
    return nc


def _prep_inputs(x, train_edge_index, pos_edge_index, neg_edge_index,
                 W1, b1, W2, b2, Wl1, bl1, Wl2, bl2):
    """Host-side sharding / index layout. Returns (in_maps, C)."""
    x = np.asarray(x, np.float32)
    ei = np.asarray(train_edge_index)
    row, col = ei[0].astype(np.int64), ei[1].astype(np.int64)
    deg = np.bincount(col, minlength=N).astype(np.float32) + SELF_LOOP_W
    dinv = (1.0 / np.sqrt(deg)).astype(np.float32)

    import ml_dtypes
    bf16 = ml_dtypes.bfloat16
    W1 = np.asarray(W1, np.float32).astype(bf16)
    W2 = np.asarray(W2, np.float32).astype(bf16)
    Wl1 = np.asarray(Wl1, np.float32).astype(bf16)
    b1 = np.asarray(b1, np.float32)
    b2 = np.asarray(b2, np.float32)
    bl1 = np.asarray(bl1, np.float32)
    Wl2 = np.asarray(Wl2, np.float32).reshape(-1)
    bl2 = np.asarray(bl2, np.float32).reshape(-1)

    # --- per-(core,tile) edge grouping ---
    core_of = col // NS
    tile_of = (col % NS) // 128
    # chunk requirement per (core, tile)
    counts = np.zeros((N_CORES, NT), np.int64)
    np.add.at(counts, (core_of, tile_of), 1)
    Cts = [int(np.ceil(counts[:, t].max() / 128.0)) for t in range(NT)]
    bases = np.cumsum([0] + Cts)[:-1]
    CHT = int(sum(Cts))

    order = np.lexsort((tile_of, core_of))
    rowp = (row // NS) * NSP + (row % NS)  # padded-global row ids
    row_s, col_s = rowp[order], col[order]
    core_s, tile_s = core_of[order], tile_of[order]
    # boundaries per (core,tile)
    grp = core_s * NT + tile_s
    starts = np.searchsorted(grp, np.arange(N_CORES * NT))
    ends = np.searchsorted(grp, np.arange(N_CORES * NT), side='right')

    tei = np.concatenate([np.asarray(pos_edge_index),
                          np.asarray(neg_edge_index)], axis=-1)
    t0_all = tei[0].astype(np.int64)
    t1_all = tei[1].astype(np.int64)
    t0_all = (t0_all // NS) * NSP + (t0_all % NS)
    t1_all = (t1_all // NS) * NSP + (t1_all % NS)

    in_maps = []
    for k in range(N_CORES):
        lo = k * NS
        esrc = np.zeros((128, CHT), np.int32)
        colloc = np.full((128, CHT), -1.0, np.float32)  # cast to bf16 at pack
        for t in range(NT):
            Ct = Cts[t]
            base = int(bases[t])
            s, e = starts[k * NT + t], ends[k * NT + t]
            ne = e - s
            assert ne <= Ct * 128, "chunk overflow"
            srcs = row_s[s:e]
            locs = (col_s[s:e] - lo - t * 128).astype(np.float32)
            full = np.zeros(Ct * 128, np.int32)
            fullc = np.full(Ct * 128, -1.0, np.float32)
            full[:ne] = srcs
            fullc[:ne] = locs
            esrc[:, base: base + Ct] = full.reshape(Ct, 128).T
            colloc[:, base: base + Ct] = fullc.reshape(Ct, 128).T  # cast below

        # head edges
        h0 = np.zeros(NHC * 128, np.int32)
        h1 = np.zeros(NHC * 128, np.int32)
        h0[:HE_CORE] = t0_all[k * HE_CORE:(k + 1) * HE_CORE]
        h1[:HE_CORE] = t1_all[k * HE_CORE:(k + 1) * HE_CORE]
        hsrc0 = h0.reshape(NHC, 128).T.copy()
        hsrc1 = h1.reshape(NHC, 128).T.copy()

        # node shard data
        xs = np.zeros((NSP, F_IN), np.float32)
        xs[:NS] = x[lo:lo + NS]
        xT = np.ascontiguousarray(xs.T).astype(bf16)
        dpm = np.zeros((128, NT), np.float32)
        dsh = np.zeros(NSP, np.float32)
        dsh[:NS] = dinv[lo:lo + NS]
        dpm[:, :] = dsh.reshape(NT, 128).T

        in_maps.append({
            "xT": xT, "dinv_pm": dpm,
            "W1": W1, "W2": W2, "Wl1": Wl1,
            "b1t": np.tile(b1[None, :], (128, 1)),
            "b2t": np.tile(b2[None, :], (128, 1)),
            "bl1t": np.tile(bl1[None, :], (128, 1)),
            "wl2bc": np.tile(Wl2[None, :], (128, 1)),
            "bl2c": np.full((128, 1), bl2[0], np.float32),
            "esrc": esrc, "colloc": colloc.astype(bf16),
            "hsrc0": hsrc0, "hsrc1": hsrc1,
        })
    return in_maps, tuple(Cts)


def _get_runner(C, in_maps):
    import jax
    from concourse import bass2jax, mybir as mb
    from jax.sharding import Mesh, PartitionSpec
    from jax.experimental.shard_map import shard_map

    key = ("runner", C)
    if key in _CACHE:
        return _CACHE[key]

    nc = _CACHE.get(("nc", C))
    if nc is None:
        nc = _build_and_compile(C)
        _CACHE[("nc", C)] = nc

    bass2jax.install_neuronx_cc_hook()
    partition_name = nc.partition_id_tensor.name if nc.partition_id_tensor else None
    in_names, out_names, out_avals, zero_outs = [], [], [], []
    for a in nc.m.functions[0].allocations:
        if not isinstance(a, mb.MemoryLocationSet):
            continue
        name = a.memorylocations[0].name
        if a.kind == "ExternalInput":
            if name != partition_name:
                in_names.append(name)
        elif a.kind == "ExternalOutput":
            out_names.append(name)
            shape = tuple(a.tensor_shape)
            dtype = mb.dt.np(a.dtype)
            out_avals.append(jax.core.ShapedArray(shape, dtype))
            zero_outs.append(np.zeros(shape, dtype))
    n_params = len(in_names)
    all_in_names = in_names + out_names + ([partition_name] if partition_name else [])

    def _body(*args):
        operands = list(args)
        if partition_name is not None:
            operands.append(bass2jax.partition_id_tensor())
        outs = bass2jax._bass_exec_p.bind(
            *operands, out_avals=tuple(out_avals), in_names=tuple(all_in_names),
            out_names=tuple(out_names), lowering_input_output_aliases=(),
            sim_require_finite=True, sim_require_nnan=True, nc=nc)
        return tuple(outs)

    devices = jax.devices()[:N_CORES]
    mesh = Mesh(np.asarray(devices), ("core",))
    in_specs = (PartitionSpec("core"),) * (n_params + len(out_names))
    out_specs = (PartitionSpec("core"),) * len(out_names)
    sharded = jax.jit(shard_map(_body, mesh=mesh, in_specs=in_specs,
                                out_specs=out_specs, check_rep=False),
                      keep_unused=True)

    def run(maps):
        concat_in = [np.concatenate([np.asarray(maps[c][nm])
                                     for c in range(N_CORES)], axis=0)
                     for nm in in_names]
        concat_zero = [np.concatenate([z] * N_CORES, axis=0) for z in zero_outs]
        outs = sharded(*concat_in, *concat_zero)
        jax.block_until_ready(outs)
        return {nm: np.asarray(outs[i]) for i, nm in enumerate(out_names)}

    _CACHE[key] = run
    return run


def kernel(**inputs) -> np.ndarray:
    in_maps, C = _prep_inputs(**inputs)
    run = _get_runner(C, in_maps)
    outs = run(in_maps)
    oh = outs["out_head"].reshape(N_CORES, 128, NHC)
    res = np.empty(HEAD_E, np.float32)
    for k in range(N_CORES):
        flat = oh[k].T.reshape(-1)  # [NHC*128] in edge order
        res[k * HE_CORE:(k + 1) * HE_CORE] = flat[:HE_CORE]
    return res



# revision 21
# speedup vs baseline: 2.6112x; 1.9460x over previous
"""GCN link-prediction kernel for 8 Trainium2 NeuronCores.

Strategy (target-sharded edges, replicated-by-AllGather node tables):
  - Nodes split into 8 contiguous shards. Each core computes its shard of
    g1 = dinv * (x @ W1) on PE, AllGather -> full table gtab1 in every
    core's HBM.
  - Train edges assigned to the core owning the TARGET node, grouped per
    128-target tile, padded to a fixed chunk count. Per 128-edge chunk:
    indirect-DMA gather of source rows, a DVE is_equal indicator matrix
    [edges x targets], and one PE matmul accumulating into PSUM.
    Self-loops (weight 2) are a per-tile extra chunk whose indicator is
    doubled.
  - Layer epilogue fuses dinv scaling, bias, relu, and the next layer's
    dense matmul (transposed via PE) so h1/h2 never round-trip to DRAM.
  - Edge head: z = h2 @ Wl1 table (64 f32), two gathers per 128-edge
    chunk, elementwise ops + free-dim reduction + sigmoid.

All float math runs on device in fp32; host only reorders/pads integer
edge indices and computes dinv (index-derived degree scaling).
"""
import sys
import os
import numpy as np

sys.path.insert(0, '/opt/trn_rl_repo')

N_CORES = 8
N = 50000
F_IN, H1, H2, H3 = 256, 256, 128, 64
SELF_LOOP_W = 2.0

NS = N // N_CORES            # 6250 nodes per shard
NT = (NS + 127) // 128       # 49 target tiles per core
NSP = NT * 128               # 6272 padded shard rows
HEAD_E = 400000
HE_CORE = HEAD_E // N_CORES  # 50000 head edges per core
NHC = (HE_CORE + 127) // 128  # 391 head chunks

_CACHE = {}
_HEAD_PERM = {}


def _build_and_compile(Cts, variant='full'):
    """Build the SPMD Bass program. Cts[t] = data-chunks for target tile t."""
    import concourse.bass as bass
    import concourse.mybir as mybir
    import concourse.tile as tile
    from concourse import bacc

    dt = mybir.dt
    Cts = list(Cts)
    CHT = sum(Cts)        # data chunks per core (self-loop via direct DMA)
    bases = np.cumsum([0] + Cts)[:-1]

    nc = bacc.Bacc("TRN2", target_bir_lowering=False, debug=False,
                   num_devices=N_CORES)

    # ---- I/O ----
    xT = nc.dram_tensor("xT", [F_IN, NSP], dt.bfloat16, kind="ExternalInput")
    dinv_pm = nc.dram_tensor("dinv_pm", [128, NT], dt.float32, kind="ExternalInput")
    W1 = nc.dram_tensor("W1", [F_IN, H1], dt.bfloat16, kind="ExternalInput")
    W2 = nc.dram_tensor("W2", [H1, H2], dt.bfloat16, kind="ExternalInput")
    Wl1 = nc.dram_tensor("Wl1", [H2, H3], dt.bfloat16, kind="ExternalInput")
    b1t = nc.dram_tensor("b1t", [128, H1], dt.float32, kind="ExternalInput")
    b2t = nc.dram_tensor("b2t", [128, H2], dt.float32, kind="ExternalInput")
    bl1t = nc.dram_tensor("bl1t", [128, H3], dt.float32, kind="ExternalInput")
    wl2bc = nc.dram_tensor("wl2bc", [128, H3], dt.float32, kind="ExternalInput")
    bl2c = nc.dram_tensor("bl2c", [128, 1], dt.float32, kind="ExternalInput")
    esrc = nc.dram_tensor("esrc", [128, CHT], dt.int32, kind="ExternalInput")
    colloc = nc.dram_tensor("colloc", [128, CHT], dt.bfloat16, kind="ExternalInput")
    hsrc0 = nc.dram_tensor("hsrc0", [128, NHC], dt.int32, kind="ExternalInput")
    hsrc1 = nc.dram_tensor("hsrc1", [128, NHC], dt.int32, kind="ExternalInput")
    bl1g = nc.dram_tensor("bl1g", [128, 8 * H3], dt.float32, kind="ExternalInput")
    wl2g = nc.dram_tensor("wl2g", [128, 8 * H3], dt.float32, kind="ExternalInput")
    out_head = nc.dram_tensor("out_head", [128, NHC], dt.float32,
                              kind="ExternalOutput")

    from concourse.masks import make_identity

    with tile.TileContext(nc) as tc:
        with tc.tile_pool(name="const", bufs=1) as cpool, \
             tc.tile_pool(name="dram", bufs=1, space="DRAM") as dpool, \
             tc.tile_pool(name="gat", bufs=40) as gat_pool, \
             tc.tile_pool(name="ind", bufs=40) as ind_pool, \
             tc.tile_pool(name="work", bufs=6) as work, \
             tc.tile_pool(name="psA", bufs=3, space="PSUM") as psA, \
             tc.tile_pool(name="psT", bufs=2, space="PSUM") as psT:

            # ---- constants / index preload ----
            ident = cpool.tile([128, 128], dt.float32)
            make_identity(nc, ident[:])
            iota_i = cpool.tile([128, 128], dt.int32)
            nc.gpsimd.iota(iota_i[:], pattern=[[1, 128]], base=0,
                           channel_multiplier=0)
            iota_f = cpool.tile([128, 128], dt.bfloat16)
            nc.vector.tensor_copy(out=iota_f[:], in_=iota_i[:])
            ident2f = cpool.tile([128, 128], dt.float32)
            nc.vector.tensor_scalar_mul(ident2f[:], ident[:], float(SELF_LOOP_W))
            ident2 = cpool.tile([128, 128], dt.bfloat16)
            nc.vector.tensor_copy(out=ident2[:], in_=ident2f[:])

            W1s = cpool.tile([128, 2 * H1], dt.bfloat16)
            nc.sync.dma_start(out=W1s[:, :H1], in_=W1[0:128, :])
            nc.sync.dma_start(out=W1s[:, H1:], in_=W1[128:256, :])
            W2s = cpool.tile([128, 2 * H2], dt.bfloat16)
            nc.sync.dma_start(out=W2s[:, :H2], in_=W2[0:128, :])
            nc.sync.dma_start(out=W2s[:, H2:], in_=W2[128:256, :])
            Wl1s = cpool.tile([128, H3], dt.bfloat16)
            nc.sync.dma_start(out=Wl1s[:], in_=Wl1[:])
            b1s = cpool.tile([128, H1], dt.float32)
            nc.sync.dma_start(out=b1s[:], in_=b1t[:])
            b2s = cpool.tile([128, H2], dt.float32)
            nc.sync.dma_start(out=b2s[:], in_=b2t[:])
            bl1s = cpool.tile([128, H3], dt.float32)
            nc.sync.dma_start(out=bl1s[:], in_=bl1t[:])
            wl2s = cpool.tile([128, H3], dt.float32)
            nc.sync.dma_start(out=wl2s[:], in_=wl2bc[:])
            bl2s = cpool.tile([128, 1], dt.float32)
            nc.sync.dma_start(out=bl2s[:], in_=bl2c[:])
            dinv_s = cpool.tile([128, NT], dt.float32)
            nc.sync.dma_start(out=dinv_s[:], in_=dinv_pm[:])
            esrc_s = cpool.tile([128, CHT], dt.int32)
            nc.sync.dma_start(out=esrc_s[:], in_=esrc[:])
            colloc_s = cpool.tile([128, CHT], dt.bfloat16)
            nc.sync.dma_start(out=colloc_s[:], in_=colloc[:])
            h0_s = cpool.tile([128, NHC], dt.int32)
            nc.sync.dma_start(out=h0_s[:], in_=hsrc0[:])
            h1_s = cpool.tile([128, NHC], dt.int32)
            nc.sync.dma_start(out=h1_s[:], in_=hsrc1[:])
            bl1gs = cpool.tile([128, 8 * H3], dt.float32)
            nc.sync.dma_start(out=bl1gs[:], in_=bl1g[:])
            wl2gs = cpool.tile([128, 8 * H3], dt.float32)
            nc.sync.dma_start(out=wl2gs[:], in_=wl2g[:])

            # ---- DRAM internals ----
            NP = N_CORES * NSP
            g1_loc = dpool.tile([NSP, H1], dt.bfloat16)
            gtab1 = dpool.tile([NP, H1], dt.bfloat16, addr_space="Shared")
            g2_loc = dpool.tile([NSP, H2], dt.bfloat16)
            gtab2 = dpool.tile([NP, H2], dt.bfloat16, addr_space="Shared")
            z_loc = dpool.tile([NSP, H3], dt.bfloat16)
            ztab = dpool.tile([NP, H3], dt.bfloat16, addr_space="Shared")
            selfb1 = cpool.tile([128, NT * H1], dt.bfloat16)
            selfb2 = cpool.tile([128, NT * H2], dt.bfloat16)
            g2stage = cpool.tile([128, NT * H2], dt.bfloat16)
            zstage = cpool.tile([128, NT * H3], dt.bfloat16)
            if variant == 'localtab':
                gtab1L = dpool.tile([N, H1], dt.bfloat16)
                gtab2L = dpool.tile([N, H2], dt.bfloat16)
                ztabL = dpool.tile([N, H3], dt.bfloat16)

            rg = [list(range(N_CORES))]

            # ================= Phase A: g1 shard =================
            xTs = cpool.tile([128, 2 * NSP], dt.bfloat16)
            nc.sync.dma_start(out=xTs[:, :NSP], in_=xT[0:128, :])
            nc.sync.dma_start(out=xTs[:, NSP:], in_=xT[128:256, :])
            for t in range(NT):
                ps = psA.tile([128, H1], dt.float32, tag="psagg")
                nc.tensor.matmul(out=ps[:],
                                 lhsT=xTs[:, t * 128:(t + 1) * 128],
                                 rhs=W1s[:, :H1], start=True, stop=False)
                nc.tensor.matmul(out=ps[:],
                                 lhsT=xTs[:, NSP + t * 128:NSP + (t + 1) * 128],
                                 rhs=W1s[:, H1:], start=False, stop=True)
                g1v = work.tile([128, H1], dt.bfloat16, tag="g1v")
                nc.vector.tensor_mul(
                    out=g1v[:], in0=ps[:],
                    in1=dinv_s[:, t:t + 1].to_broadcast([128, H1]))
                nc.sync.dma_start(out=g1_loc[t * 128: (t + 1) * 128, :],
                                  in_=g1v[:, :])
            if variant != 'nocoll':
                nc.gpsimd.collective_compute(
                    "AllGather", mybir.AluOpType.bypass, replica_groups=rg,
                    ins=[g1_loc.opt()], outs=[gtab1.opt()])
            if variant == 'localtab':
                nc.sync.dma_start(out=gtab1L[:, :], in_=gtab1[:, :])
                gtab1 = gtab1L

            # ============ Layer helpers ============
            def agg_layer(gtab, F, selfb):
                """Yields per-tile psum [128, F]: self-loop (x2, from the
                SBUF-resident self table) + Cts[t] gathered chunks."""
                for t in range(NT):
                    if variant == 'dmaonly':
                        ps = None
                    else:
                        ps = psA.tile([128, F], dt.float32, tag="psagg")
                        nc.tensor.matmul(out=ps[:], lhsT=ident2[:],
                                         rhs=selfb[:, t * F:(t + 1) * F],
                                         start=True,
                                         stop=(variant in ('nomm', 'noagg')))
                    for c in range(0 if variant == 'noagg' else Cts[t]):
                        j = int(bases[t]) + c
                        g = gat_pool.tile([128, F], dt.bfloat16, tag="gath")
                        nc.gpsimd.indirect_dma_start(
                            out=g[:], out_offset=None, in_=gtab[:],
                            in_offset=bass.IndirectOffsetOnAxis(
                                ap=esrc_s[:, j:j + 1], axis=0))
                        if variant == 'dmaonly':
                            continue
                        if variant in ('noind', 'nomm'):
                            ind = iota_f
                        else:
                            ind = ind_pool.tile([128, 128], dt.bfloat16,
                                                tag="ind")
                            nc.vector.tensor_tensor(
                                out=ind[:],
                                in0=colloc_s[:, j:j + 1].to_broadcast([128, 128]),
                                in1=iota_f[:], op=mybir.AluOpType.is_equal)
                        if variant != 'nomm':
                            nc.tensor.matmul(out=ps[:], lhsT=ind[:], rhs=g[:],
                                             start=False,
                                             stop=(c == Cts[t] - 1))
                    yield t, ps

            # ============ Layer 1 + fused g2 ============
            nc.sync.dma_start(
                out=selfb1[:].rearrange("p (t f) -> p t f", t=NT),
                in_=g1_loc[:].rearrange("(t p) f -> p t f", p=128))
            for t, ps in agg_layer(gtab1, H1, selfb1):
                rows = min(128, NS - t * 128)
                if variant == 'dmaonly':
                    g2v = work.tile([128, H2], dt.bfloat16, tag="g2v")
                    nc.sync.dma_start(out=g2_loc[t * 128: t * 128 + rows, :],
                                      in_=g2v[:rows, :])
                    continue
                dv = dinv_s[:, t:t + 1]
                h1v = work.tile([128, H1], dt.float32, tag="h1v")
                nc.vector.tensor_mul(out=h1v[:], in0=ps[:],
                                     in1=dv.to_broadcast([128, H1]))
                nc.vector.tensor_add(out=h1v[:], in0=h1v[:], in1=b1s[:])
                nc.scalar.activation(out=h1v[:], in_=h1v[:],
                                     func=mybir.ActivationFunctionType.Relu)
                nc.vector.tensor_mul(out=h1v[:], in0=h1v[:],
                                     in1=dv.to_broadcast([128, H1]))
                # transpose h1d -> [feat, rows], then g2 = h1d @ W2
                g2ps = psA.tile([128, H2], dt.float32, tag="pssm")
                tpss = []
                for fb in range(2):
                    tp = psT.tile([128, 128], dt.float32, tag="tp")
                    nc.tensor.transpose(out=tp[:],
                                        in_=h1v[:, fb * 128:(fb + 1) * 128],
                                        identity=ident[:])
                    tps = work.tile([128, 128], dt.bfloat16, tag=f"tps{fb}")
                    nc.vector.tensor_copy(out=tps[:], in_=tp[:])
                    tpss.append(tps)
                for fb in range(2):
                    nc.tensor.matmul(out=g2ps[:], lhsT=tpss[fb][:],
                                     rhs=W2s[:, fb * H2:(fb + 1) * H2],
                                     start=(fb == 0), stop=(fb == 1))
                nc.vector.tensor_copy(out=g2stage[:, t * H2:(t + 1) * H2],
                                       in_=g2ps[:])
            nc.sync.dma_start(
                out=g2_loc[:].rearrange("(t p) f -> p t f", p=128),
                in_=g2stage[:].rearrange("p (t f) -> p t f", t=NT))
            if variant != 'nocoll':
                nc.gpsimd.collective_compute(
                    "AllGather", mybir.AluOpType.bypass, replica_groups=rg,
                    ins=[g2_loc.opt()], outs=[gtab2.opt()])
            if variant == 'localtab':
                nc.sync.dma_start(out=gtab2L[:, :], in_=gtab2[:, :])
                gtab2 = gtab2L

            # ============ Layer 2 + fused z ============
            nc.sync.dma_start(
                out=selfb2[:].rearrange("p (t f) -> p t f", t=NT),
                in_=g2_loc[:].rearrange("(t p) f -> p t f", p=128))
            for t, ps in agg_layer(gtab2, H2, selfb2):
                rows = min(128, NS - t * 128)
                if variant == 'dmaonly':
                    zv = work.tile([128, H3], dt.bfloat16, tag="zv")
                    nc.sync.dma_start(out=z_loc[t * 128: t * 128 + rows, :],
                                      in_=zv[:rows, :])
                    continue
                dv = dinv_s[:, t:t + 1]
                h2v = work.tile([128, H2], dt.float32, tag="h2v")
                nc.vector.tensor_mul(out=h2v[:], in0=ps[:],
                                     in1=dv.to_broadcast([128, H2]))
                nc.vector.tensor_add(out=h2v[:], in0=h2v[:], in1=b2s[:])
                tp = psT.tile([128, 128], dt.float32, tag="tp")
                nc.tensor.transpose(out=tp[:], in_=h2v[:], identity=ident[:])
                tps = work.tile([128, 128], dt.bfloat16, tag="tps")
                nc.vector.tensor_copy(out=tps[:], in_=tp[:])
                zps = psA.tile([128, H3], dt.float32, tag="pssm")
                nc.tensor.matmul(out=zps[:], lhsT=tps[:], rhs=Wl1s[:],
                                 start=True, stop=True)
                nc.vector.tensor_copy(out=zstage[:, t * H3:(t + 1) * H3],
                                       in_=zps[:])
            nc.sync.dma_start(
                out=z_loc[:].rearrange("(t p) f -> p t f", p=128),
                in_=zstage[:].rearrange("p (t f) -> p t f", t=NT))
            if variant != 'nocoll':
                nc.gpsimd.collective_compute(
                    "AllGather", mybir.AluOpType.bypass, replica_groups=rg,
                    ins=[z_loc.opt()], outs=[ztab.opt()])
            if variant == 'localtab':
                nc.sync.dma_start(out=ztabL[:, :], in_=ztab[:, :])
                ztab = ztabL

            # ============ Edge head ============
            out_sb = cpool.tile([128, NHC], dt.float32)
            if variant in ('nohead', 'dmaonly'):
                nc.gpsimd.memset(out_sb[:], 0)
            G = 8
            NG = (NHC + G - 1) // G  # 49 groups; last group has 7 chunks
            for g in range(0 if variant == 'nohead' else NG):
                c0 = g * G
                gc = min(G, NHC - c0)
                r0 = gat_pool.tile([128, G * H3], dt.bfloat16, tag="hg0",
                                   bufs=6)
                r1 = gat_pool.tile([128, G * H3], dt.bfloat16, tag="hg1",
                                   bufs=6)
                for c in range(gc):
                    nc.gpsimd.indirect_dma_start(
                        out=r0[:, c * H3:(c + 1) * H3], out_offset=None,
                        in_=ztab[:],
                        in_offset=bass.IndirectOffsetOnAxis(
                            ap=h0_s[:, c0 + c:c0 + c + 1], axis=0))
                    nc.gpsimd.indirect_dma_start(
                        out=r1[:, c * H3:(c + 1) * H3], out_offset=None,
                        in_=ztab[:],
                        in_offset=bass.IndirectOffsetOnAxis(
                            ap=h1_s[:, c0 + c:c0 + c + 1], axis=0))
                if variant == 'dmaonly':
                    continue
                W = gc * H3
                e1 = work.tile([128, G * H3], dt.float32, tag="e1", bufs=4)
                nc.vector.tensor_add(out=e1[:, :W], in0=r0[:, :W],
                                     in1=r1[:, :W])
                nc.vector.tensor_add(out=e1[:, :W], in0=e1[:, :W],
                                     in1=bl1gs[:, :W])
                nc.scalar.activation(out=e1[:, :W], in_=e1[:, :W],
                                     func=mybir.ActivationFunctionType.Relu)
                nc.vector.tensor_mul(out=e1[:, :W], in0=e1[:, :W],
                                     in1=wl2gs[:, :W])
                sc = work.tile([128, G], dt.float32, tag="sc", bufs=4)
                nc.vector.reduce_sum(
                    out=sc[:, :gc],
                    in_=e1[:, :W].rearrange("p (g f) -> p g f", g=gc),
                    axis=mybir.AxisListType.X)
                nc.scalar.activation(out=out_sb[:, c0:c0 + gc],
                                     in_=sc[:, :gc],
                                     func=mybir.ActivationFunctionType.Sigmoid,
                                     bias=bl2s[:])
            nc.sync.dma_start(out=out_head[:], in_=out_sb[:])

    nc.compile()
    return nc


def _prep_inputs(x, train_edge_index, pos_edge_index, neg_edge_index,
                 W1, b1, W2, b2, Wl1, bl1, Wl2, bl2):
    """Host-side sharding / index layout. Returns (in_maps, C)."""
    x = np.asarray(x, np.float32)
    ei = np.asarray(train_edge_index)
    row, col = ei[0].astype(np.int64), ei[1].astype(np.int64)
    deg = np.bincount(col, minlength=N).astype(np.float32) + SELF_LOOP_W
    dinv = (1.0 / np.sqrt(deg)).astype(np.float32)

    import ml_dtypes
    bf16 = ml_dtypes.bfloat16
    W1 = np.asarray(W1, np.float32).astype(bf16)
    W2 = np.asarray(W2, np.float32).astype(bf16)
    Wl1 = np.asarray(Wl1, np.float32).astype(bf16)
    b1 = np.asarray(b1, np.float32)
    b2 = np.asarray(b2, np.float32)
    bl1 = np.asarray(bl1, np.float32)
    Wl2 = np.asarray(Wl2, np.float32).reshape(-1)
    bl2 = np.asarray(bl2, np.float32).reshape(-1)

    # --- per-(core,tile) edge grouping ---
    core_of = col // NS
    tile_of = (col % NS) // 128
    # chunk requirement per (core, tile)
    counts = np.zeros((N_CORES, NT), np.int64)
    np.add.at(counts, (core_of, tile_of), 1)
    Cts = [int(np.ceil(counts[:, t].max() / 128.0)) for t in range(NT)]
    bases = np.cumsum([0] + Cts)[:-1]
    CHT = int(sum(Cts))

    order = np.lexsort((tile_of, core_of))
    rowp = (row // NS) * NSP + (row % NS)  # padded-global row ids
    row_s, col_s = rowp[order], col[order]
    core_s, tile_s = core_of[order], tile_of[order]
    # boundaries per (core,tile)
    grp = core_s * NT + tile_s
    starts = np.searchsorted(grp, np.arange(N_CORES * NT))
    ends = np.searchsorted(grp, np.arange(N_CORES * NT), side='right')

    tei = np.concatenate([np.asarray(pos_edge_index),
                          np.asarray(neg_edge_index)], axis=-1)
    t0_all = tei[0].astype(np.int64)
    t1_all = tei[1].astype(np.int64)
    t0_all = (t0_all // NS) * NSP + (t0_all % NS)
    t1_all = (t1_all // NS) * NSP + (t1_all % NS)

    in_maps = []
    for k in range(N_CORES):
        lo = k * NS
        esrc = np.zeros((128, CHT), np.int32)
        colloc = np.full((128, CHT), -1.0, np.float32)  # cast to bf16 at pack
        for t in range(NT):
            Ct = Cts[t]
            base = int(bases[t])
            s, e = starts[k * NT + t], ends[k * NT + t]
            ne = e - s
            assert ne <= Ct * 128, "chunk overflow"
            srcs = row_s[s:e]
            locs = (col_s[s:e] - lo - t * 128).astype(np.float32)
            so = np.argsort(srcs, kind='stable')
            srcs, locs = srcs[so], locs[so]
            full = np.zeros(Ct * 128, np.int32)
            fullc = np.full(Ct * 128, -1.0, np.float32)
            full[:ne] = srcs
            fullc[:ne] = locs
            esrc[:, base: base + Ct] = full.reshape(Ct, 128).T
            colloc[:, base: base + Ct] = fullc.reshape(Ct, 128).T  # cast below

        # head edges: sort by i-row for gather locality; remember perm
        h0 = np.zeros(NHC * 128, np.int32)
        h1 = np.zeros(NHC * 128, np.int32)
        t0k = t0_all[k * HE_CORE:(k + 1) * HE_CORE]
        t1k = t1_all[k * HE_CORE:(k + 1) * HE_CORE]
        hperm = np.argsort(t0k, kind='stable')
        _HEAD_PERM[k] = hperm
        h0[:HE_CORE] = t0k[hperm]
        h1[:HE_CORE] = t1k[hperm]
        hsrc0 = h0.reshape(NHC, 128).T.copy()
        hsrc1 = h1.reshape(NHC, 128).T.copy()

        # node shard data
        xs = np.zeros((NSP, F_IN), np.float32)
        xs[:NS] = x[lo:lo + NS]
        xT = np.ascontiguousarray(xs.T).astype(bf16)
        dpm = np.zeros((128, NT), np.float32)
        dsh = np.zeros(NSP, np.float32)
        dsh[:NS] = dinv[lo:lo + NS]
        dpm[:, :] = dsh.reshape(NT, 128).T

        in_maps.append({
            "xT": xT, "dinv_pm": dpm,
            "W1": W1, "W2": W2, "Wl1": Wl1,
            "b1t": np.tile(b1[None, :], (128, 1)),
            "b2t": np.tile(b2[None, :], (128, 1)),
            "bl1t": np.tile(bl1[None, :], (128, 1)),
            "wl2bc": np.tile(Wl2[None, :], (128, 1)),
            "bl2c": np.full((128, 1), bl2[0], np.float32),
            "esrc": esrc, "colloc": colloc.astype(bf16),
            "bl1g": np.tile(bl1[None, :], (128, 8)),
            "wl2g": np.tile(Wl2[None, :], (128, 8)),
            "hsrc0": hsrc0, "hsrc1": hsrc1,
        })
    return in_maps, tuple(Cts)


def _get_runner(C, in_maps):
    import jax
    from concourse import bass2jax, mybir as mb
    from jax.sharding import Mesh, PartitionSpec
    from jax.experimental.shard_map import shard_map

    key = ("runner", C)
    if key in _CACHE:
        return _CACHE[key]

    nc = _CACHE.get(("nc", C))
    if nc is None:
        nc = _build_and_compile(C)
        _CACHE[("nc", C)] = nc

    bass2jax.install_neuronx_cc_hook()
    partition_name = nc.partition_id_tensor.name if nc.partition_id_tensor else None
    in_names, out_names, out_avals, zero_outs = [], [], [], []
    for a in nc.m.functions[0].allocations:
        if not isinstance(a, mb.MemoryLocationSet):
            continue
        name = a.memorylocations[0].name
        if a.kind == "ExternalInput":
            if name != partition_name:
                in_names.append(name)
        elif a.kind == "ExternalOutput":
            out_names.append(name)
            shape = tuple(a.tensor_shape)
            dtype = mb.dt.np(a.dtype)
            out_avals.append(jax.core.ShapedArray(shape, dtype))
            zero_outs.append(np.zeros(shape, dtype))
    n_params = len(in_names)
    all_in_names = in_names + out_names + ([partition_name] if partition_name else [])

    def _body(*args):
        operands = list(args)
        if partition_name is not None:
            operands.append(bass2jax.partition_id_tensor())
        outs = bass2jax._bass_exec_p.bind(
            *operands, out_avals=tuple(out_avals), in_names=tuple(all_in_names),
            out_names=tuple(out_names), lowering_input_output_aliases=(),
            sim_require_finite=True, sim_require_nnan=True, nc=nc)
        return tuple(outs)

    devices = jax.devices()[:N_CORES]
    mesh = Mesh(np.asarray(devices), ("core",))
    in_specs = (PartitionSpec("core"),) * (n_params + len(out_names))
    out_specs = (PartitionSpec("core"),) * len(out_names)
    sharded = jax.jit(shard_map(_body, mesh=mesh, in_specs=in_specs,
                                out_specs=out_specs, check_rep=False),
                      keep_unused=True)

    def run(maps):
        concat_in = [np.concatenate([np.asarray(maps[c][nm])
                                     for c in range(N_CORES)], axis=0)
                     for nm in in_names]
        concat_zero = [np.concatenate([z] * N_CORES, axis=0) for z in zero_outs]
        outs = sharded(*concat_in, *concat_zero)
        jax.block_until_ready(outs)
        return {nm: np.asarray(outs[i]) for i, nm in enumerate(out_names)}

    _CACHE[key] = run
    return run


def kernel(**inputs) -> np.ndarray:
    in_maps, C = _prep_inputs(**inputs)
    run = _get_runner(C, in_maps)
    outs = run(in_maps)
    oh = outs["out_head"].reshape(N_CORES, 128, NHC)
    res = np.empty(HEAD_E, np.float32)
    for k in range(N_CORES):
        flat = oh[k].T.reshape(-1)  # [NHC*128] in sorted-edge order
        seg = np.empty(HE_CORE, np.float32)
        seg[_HEAD_PERM[k]] = flat[:HE_CORE]
        res[k * HE_CORE:(k + 1) * HE_CORE] = seg
    return res



# revision 23
# speedup vs baseline: 3.5846x; 1.3728x over previous
"""GCN link-prediction kernel for 8 Trainium2 NeuronCores.

Strategy (target-sharded edges, replicated-by-AllGather node tables):
  - Nodes split into 8 contiguous shards. Each core computes its shard of
    g1 = dinv * (x @ W1) on PE, AllGather -> full table gtab1 in every
    core's HBM.
  - Train edges assigned to the core owning the TARGET node, grouped per
    128-target tile, padded to a fixed chunk count. Per 128-edge chunk:
    indirect-DMA gather of source rows, a DVE is_equal indicator matrix
    [edges x targets], and one PE matmul accumulating into PSUM.
    Self-loops (weight 2) are a per-tile extra chunk whose indicator is
    doubled.
  - Layer epilogue fuses dinv scaling, bias, relu, and the next layer's
    dense matmul (transposed via PE) so h1/h2 never round-trip to DRAM.
  - Edge head: z = h2 @ Wl1 table (64 f32), two gathers per 128-edge
    chunk, elementwise ops + free-dim reduction + sigmoid.

All float math runs on device in fp32; host only reorders/pads integer
edge indices and computes dinv (index-derived degree scaling).
"""
import sys
import os
import numpy as np

sys.path.insert(0, '/opt/trn_rl_repo')

N_CORES = 8
N = 50000
F_IN, H1, H2, H3 = 256, 256, 128, 64
SELF_LOOP_W = 2.0

NS = N // N_CORES            # 6250 nodes per shard
NT = (NS + 127) // 128       # 49 target tiles per core
NSP = NT * 128               # 6272 padded shard rows
HEAD_E = 400000
HE_CORE = HEAD_E // N_CORES  # 50000 head edges per core
NHC = (HE_CORE + 127) // 128  # 391 head chunks

_CACHE = {}
_HEAD_PERM = {}


def _build_and_compile(Cab, variant='full'):
    """Build the SPMD Bass program. Cab=(CtsA,CtsB) per-tile chunk counts,
    split by source shard-row half (A: rows 0:3200 of each shard)."""
    import concourse.bass as bass
    import concourse.mybir as mybir
    import concourse.tile as tile
    from concourse import bacc

    dt = mybir.dt
    CtsA, CtsB = [list(c) for c in Cab]
    Cts = [a + b for a, b in zip(CtsA, CtsB)]
    CHT = sum(Cts)
    bases = np.cumsum([0] + Cts)[:-1]
    SRA = 3200  # shard-row split: tiles 0..24 -> A, 25..48 -> B

    nc = bacc.Bacc("TRN2", target_bir_lowering=False, debug=False,
                   num_devices=N_CORES)

    # ---- I/O ----
    xT = nc.dram_tensor("xT", [F_IN, NSP], dt.bfloat16, kind="ExternalInput")
    dinv_pm = nc.dram_tensor("dinv_pm", [128, NT], dt.float32, kind="ExternalInput")
    W1 = nc.dram_tensor("W1", [F_IN, H1], dt.bfloat16, kind="ExternalInput")
    W2 = nc.dram_tensor("W2", [H1, H2], dt.bfloat16, kind="ExternalInput")
    Wl1 = nc.dram_tensor("Wl1", [H2, H3], dt.bfloat16, kind="ExternalInput")
    b1t = nc.dram_tensor("b1t", [128, H1], dt.float32, kind="ExternalInput")
    b2t = nc.dram_tensor("b2t", [128, H2], dt.float32, kind="ExternalInput")
    bl1t = nc.dram_tensor("bl1t", [128, H3], dt.float32, kind="ExternalInput")
    wl2bc = nc.dram_tensor("wl2bc", [128, H3], dt.float32, kind="ExternalInput")
    bl2c = nc.dram_tensor("bl2c", [128, 1], dt.float32, kind="ExternalInput")
    esrc = nc.dram_tensor("esrc", [128, CHT], dt.int32, kind="ExternalInput")
    esrc2 = nc.dram_tensor("esrc2", [128, CHT], dt.int32, kind="ExternalInput")
    colloc = nc.dram_tensor("colloc", [128, CHT], dt.bfloat16, kind="ExternalInput")
    hsrc0 = nc.dram_tensor("hsrc0", [128, NHC], dt.int32, kind="ExternalInput")
    hsrc1 = nc.dram_tensor("hsrc1", [128, NHC], dt.int32, kind="ExternalInput")
    bl1g = nc.dram_tensor("bl1g", [128, 8 * H3], dt.float32, kind="ExternalInput")
    wl2g = nc.dram_tensor("wl2g", [128, 8 * H3], dt.float32, kind="ExternalInput")
    out_head = nc.dram_tensor("out_head", [128, NHC], dt.float32,
                              kind="ExternalOutput")

    from concourse.masks import make_identity

    with tile.TileContext(nc) as tc:
        with tc.tile_pool(name="const", bufs=1) as cpool, \
             tc.tile_pool(name="dram", bufs=1, space="DRAM") as dpool, \
             tc.tile_pool(name="gat", bufs=40) as gat_pool, \
             tc.tile_pool(name="ind", bufs=40) as ind_pool, \
             tc.tile_pool(name="work", bufs=6) as work, \
             tc.tile_pool(name="psA", bufs=3, space="PSUM") as psA, \
             tc.tile_pool(name="psT", bufs=2, space="PSUM") as psT:

            # ---- constants / index preload ----
            ident = cpool.tile([128, 128], dt.float32)
            make_identity(nc, ident[:])
            iota_i = cpool.tile([128, 128], dt.int32)
            nc.gpsimd.iota(iota_i[:], pattern=[[1, 128]], base=0,
                           channel_multiplier=0)
            iota_f = cpool.tile([128, 128], dt.bfloat16)
            nc.vector.tensor_copy(out=iota_f[:], in_=iota_i[:])
            ident2f = cpool.tile([128, 128], dt.float32)
            nc.vector.tensor_scalar_mul(ident2f[:], ident[:], float(SELF_LOOP_W))
            ident2 = cpool.tile([128, 128], dt.bfloat16)
            nc.vector.tensor_copy(out=ident2[:], in_=ident2f[:])

            W1s = cpool.tile([128, 2 * H1], dt.bfloat16)
            nc.sync.dma_start(out=W1s[:, :H1], in_=W1[0:128, :])
            nc.sync.dma_start(out=W1s[:, H1:], in_=W1[128:256, :])
            W2s = cpool.tile([128, 2 * H2], dt.bfloat16)
            nc.sync.dma_start(out=W2s[:, :H2], in_=W2[0:128, :])
            nc.sync.dma_start(out=W2s[:, H2:], in_=W2[128:256, :])
            Wl1s = cpool.tile([128, H3], dt.bfloat16)
            nc.sync.dma_start(out=Wl1s[:], in_=Wl1[:])
            b1s = cpool.tile([128, H1], dt.float32)
            nc.sync.dma_start(out=b1s[:], in_=b1t[:])
            b2s = cpool.tile([128, H2], dt.float32)
            nc.sync.dma_start(out=b2s[:], in_=b2t[:])
            bl1s = cpool.tile([128, H3], dt.float32)
            nc.sync.dma_start(out=bl1s[:], in_=bl1t[:])
            wl2s = cpool.tile([128, H3], dt.float32)
            nc.sync.dma_start(out=wl2s[:], in_=wl2bc[:])
            bl2s = cpool.tile([128, 1], dt.float32)
            nc.sync.dma_start(out=bl2s[:], in_=bl2c[:])
            dinv_s = cpool.tile([128, NT], dt.float32)
            nc.sync.dma_start(out=dinv_s[:], in_=dinv_pm[:])
            esrc_s = cpool.tile([128, CHT], dt.int32)
            nc.sync.dma_start(out=esrc_s[:], in_=esrc[:])
            esrc2_s = cpool.tile([128, CHT], dt.int32)
            nc.sync.dma_start(out=esrc2_s[:], in_=esrc2[:])
            colloc_s = cpool.tile([128, CHT], dt.bfloat16)
            nc.sync.dma_start(out=colloc_s[:], in_=colloc[:])
            h0_s = cpool.tile([128, NHC], dt.int32)
            nc.sync.dma_start(out=h0_s[:], in_=hsrc0[:])
            h1_s = cpool.tile([128, NHC], dt.int32)
            nc.sync.dma_start(out=h1_s[:], in_=hsrc1[:])
            bl1gs = cpool.tile([128, 8 * H3], dt.float32)
            nc.sync.dma_start(out=bl1gs[:], in_=bl1g[:])
            wl2gs = cpool.tile([128, 8 * H3], dt.float32)
            nc.sync.dma_start(out=wl2gs[:], in_=wl2g[:])

            # ---- DRAM internals ----
            NP = N_CORES * NSP
            g1_loc = dpool.tile([NSP, H1], dt.bfloat16)
            gtab1 = dpool.tile([NP, H1], dt.bfloat16, addr_space="Shared")
            g2_locA = dpool.tile([SRA, H2], dt.bfloat16)
            g2_locB = dpool.tile([NSP - SRA, H2], dt.bfloat16)
            gtab2A = dpool.tile([N_CORES * SRA, H2], dt.bfloat16,
                                addr_space="Shared")
            gtab2B = dpool.tile([N_CORES * (NSP - SRA), H2], dt.bfloat16,
                                addr_space="Shared")
            z_loc = dpool.tile([NSP, H3], dt.bfloat16)
            ztab = dpool.tile([NP, H3], dt.bfloat16, addr_space="Shared")
            selfb1 = cpool.tile([128, NT * H1], dt.bfloat16)
            selfb2 = cpool.tile([128, NT * H2], dt.bfloat16)
            g2stage = cpool.tile([128, NT * H2], dt.bfloat16)
            zstage = cpool.tile([128, NT * H3], dt.bfloat16)
            if variant == 'localtab':
                gtab1L = dpool.tile([N, H1], dt.bfloat16)
                ztabL = dpool.tile([N, H3], dt.bfloat16)

            rg = [list(range(N_CORES))]

            # ================= Phase A: g1 shard =================
            xTs = cpool.tile([128, 2 * NSP], dt.bfloat16)
            nc.sync.dma_start(out=xTs[:, :NSP], in_=xT[0:128, :])
            nc.sync.dma_start(out=xTs[:, NSP:], in_=xT[128:256, :])
            for t in range(NT):
                ps = psA.tile([128, H1], dt.float32, tag="psagg")
                nc.tensor.matmul(out=ps[:],
                                 lhsT=xTs[:, t * 128:(t + 1) * 128],
                                 rhs=W1s[:, :H1], start=True, stop=False)
                nc.tensor.matmul(out=ps[:],
                                 lhsT=xTs[:, NSP + t * 128:NSP + (t + 1) * 128],
                                 rhs=W1s[:, H1:], start=False, stop=True)
                g1v = work.tile([128, H1], dt.bfloat16, tag="g1v")
                nc.vector.tensor_mul(
                    out=g1v[:], in0=ps[:],
                    in1=dinv_s[:, t:t + 1].to_broadcast([128, H1]))
                nc.sync.dma_start(out=g1_loc[t * 128: (t + 1) * 128, :],
                                  in_=g1v[:, :])
            if variant != 'nocoll':
                nc.gpsimd.collective_compute(
                    "AllGather", mybir.AluOpType.bypass, replica_groups=rg,
                    ins=[g1_loc.opt()], outs=[gtab1.opt()])
            if variant == 'localtab':
                nc.sync.dma_start(out=gtab1L[:, :], in_=gtab1[:, :])
                gtab1 = gtab1L

            # ============ Layer helpers ============
            def agg_layer(tabs, idx_t, F, selfb):
                """Yields per-tile psum [128, F]: self-loop (x2, from the
                SBUF-resident self table) + Cts[t] gathered chunks. tabs is
                (tabA, tabB) per source-half (tabB None -> single table)."""
                tabA, tabB = tabs
                for t in range(NT):
                    if variant == 'dmaonly':
                        ps = None
                    else:
                        ps = psA.tile([128, F], dt.float32, tag="psagg")
                        nc.tensor.matmul(out=ps[:], lhsT=ident2[:],
                                         rhs=selfb[:, t * F:(t + 1) * F],
                                         start=True,
                                         stop=(variant in ('nomm', 'noagg')))
                    for c in range(0 if variant == 'noagg' else Cts[t]):
                        j = int(bases[t]) + c
                        src_tab = tabA if (tabB is None or c < CtsA[t]) \
                            else tabB
                        g = gat_pool.tile([128, F], dt.bfloat16, tag="gath")
                        nc.gpsimd.indirect_dma_start(
                            out=g[:], out_offset=None, in_=src_tab[:],
                            in_offset=bass.IndirectOffsetOnAxis(
                                ap=idx_t[:, j:j + 1], axis=0))
                        if variant == 'dmaonly':
                            continue
                        if variant in ('noind', 'nomm'):
                            ind = iota_f
                        else:
                            ind = ind_pool.tile([128, 128], dt.bfloat16,
                                                tag="ind")
                            nc.vector.tensor_tensor(
                                out=ind[:],
                                in0=colloc_s[:, j:j + 1].to_broadcast([128, 128]),
                                in1=iota_f[:], op=mybir.AluOpType.is_equal)
                        if variant != 'nomm':
                            nc.tensor.matmul(out=ps[:], lhsT=ind[:], rhs=g[:],
                                             start=False,
                                             stop=(c == Cts[t] - 1))
                    yield t, ps

            # ============ Layer 1 + fused g2 ============
            nc.sync.dma_start(
                out=selfb1[:].rearrange("p (t f) -> p t f", t=NT),
                in_=g1_loc[:].rearrange("(t p) f -> p t f", p=128))
            for t, ps in agg_layer((gtab1, None), esrc_s, H1, selfb1):
                rows = min(128, NS - t * 128)
                if variant == 'dmaonly':
                    g2v = work.tile([128, H2], dt.bfloat16, tag="g2v")
                    nc.sync.dma_start(out=g2_loc[t * 128: t * 128 + rows, :],
                                      in_=g2v[:rows, :])
                    continue
                dv = dinv_s[:, t:t + 1]
                h1v = work.tile([128, H1], dt.float32, tag="h1v")
                nc.vector.tensor_mul(out=h1v[:], in0=ps[:],
                                     in1=dv.to_broadcast([128, H1]))
                nc.vector.tensor_add(out=h1v[:], in0=h1v[:], in1=b1s[:])
                nc.scalar.activation(out=h1v[:], in_=h1v[:],
                                     func=mybir.ActivationFunctionType.Relu)
                nc.vector.tensor_mul(out=h1v[:], in0=h1v[:],
                                     in1=dv.to_broadcast([128, H1]))
                # transpose h1d -> [feat, rows], then g2 = h1d @ W2
                g2ps = psA.tile([128, H2], dt.float32, tag="pssm")
                tpss = []
                for fb in range(2):
                    tp = psT.tile([128, 128], dt.float32, tag="tp")
                    nc.tensor.transpose(out=tp[:],
                                        in_=h1v[:, fb * 128:(fb + 1) * 128],
                                        identity=ident[:])
                    tps = work.tile([128, 128], dt.bfloat16, tag=f"tps{fb}")
                    nc.vector.tensor_copy(out=tps[:], in_=tp[:])
                    tpss.append(tps)
                for fb in range(2):
                    nc.tensor.matmul(out=g2ps[:], lhsT=tpss[fb][:],
                                     rhs=W2s[:, fb * H2:(fb + 1) * H2],
                                     start=(fb == 0), stop=(fb == 1))
                nc.vector.tensor_copy(out=g2stage[:, t * H2:(t + 1) * H2],
                                       in_=g2ps[:])
                if t == 24:
                    # first 25 tiles done -> ship half A, overlap AG under
                    # the remaining 24 tiles
                    nc.sync.dma_start(
                        out=g2_locA[:].rearrange("(t p) f -> p t f", p=128),
                        in_=g2stage[:, :25 * H2].rearrange(
                            "p (t f) -> p t f", t=25))
                    if variant != 'nocoll':
                        nc.gpsimd.collective_compute(
                            "AllGather", mybir.AluOpType.bypass,
                            replica_groups=rg,
                            ins=[g2_locA.opt()], outs=[gtab2A.opt()])
            nc.sync.dma_start(
                out=g2_locB[:].rearrange("(t p) f -> p t f", p=128),
                in_=g2stage[:, 25 * H2:].rearrange("p (t f) -> p t f", t=24))
            if variant != 'nocoll':
                nc.gpsimd.collective_compute(
                    "AllGather", mybir.AluOpType.bypass, replica_groups=rg,
                    ins=[g2_locB.opt()], outs=[gtab2B.opt()])

            # ============ Layer 2 + fused z ============
            nc.sync.dma_start(
                out=selfb2[:, :25 * H2].rearrange("p (t f) -> p t f", t=25),
                in_=g2_locA[:].rearrange("(t p) f -> p t f", p=128))
            nc.sync.dma_start(
                out=selfb2[:, 25 * H2:].rearrange("p (t f) -> p t f", t=24),
                in_=g2_locB[:].rearrange("(t p) f -> p t f", p=128))
            for t, ps in agg_layer((gtab2A, gtab2B), esrc2_s, H2, selfb2):
                rows = min(128, NS - t * 128)
                if variant == 'dmaonly':
                    zv = work.tile([128, H3], dt.bfloat16, tag="zv")
                    nc.sync.dma_start(out=z_loc[t * 128: t * 128 + rows, :],
                                      in_=zv[:rows, :])
                    continue
                dv = dinv_s[:, t:t + 1]
                h2v = work.tile([128, H2], dt.float32, tag="h2v")
                nc.vector.tensor_mul(out=h2v[:], in0=ps[:],
                                     in1=dv.to_broadcast([128, H2]))
                nc.vector.tensor_add(out=h2v[:], in0=h2v[:], in1=b2s[:])
                tp = psT.tile([128, 128], dt.float32, tag="tp")
                nc.tensor.transpose(out=tp[:], in_=h2v[:], identity=ident[:])
                tps = work.tile([128, 128], dt.bfloat16, tag="tps")
                nc.vector.tensor_copy(out=tps[:], in_=tp[:])
                zps = psA.tile([128, H3], dt.float32, tag="pssm")
                nc.tensor.matmul(out=zps[:], lhsT=tps[:], rhs=Wl1s[:],
                                 start=True, stop=True)
                nc.vector.tensor_copy(out=zstage[:, t * H3:(t + 1) * H3],
                                       in_=zps[:])
            nc.sync.dma_start(
                out=z_loc[:].rearrange("(t p) f -> p t f", p=128),
                in_=zstage[:].rearrange("p (t f) -> p t f", t=NT))
            if variant != 'nocoll':
                nc.gpsimd.collective_compute(
                    "AllGather", mybir.AluOpType.bypass, replica_groups=rg,
                    ins=[z_loc.opt()], outs=[ztab.opt()])
            if variant == 'localtab':
                nc.sync.dma_start(out=ztabL[:, :], in_=ztab[:, :])
                ztab = ztabL

            # ============ Edge head ============
            out_sb = cpool.tile([128, NHC], dt.float32)
            if variant in ('nohead', 'dmaonly'):
                nc.gpsimd.memset(out_sb[:], 0)
            G = 8
            NG = (NHC + G - 1) // G  # 49 groups; last group has 7 chunks
            for g in range(0 if variant == 'nohead' else NG):
                c0 = g * G
                gc = min(G, NHC - c0)
                r0 = gat_pool.tile([128, G * H3], dt.bfloat16, tag="hg0",
                                   bufs=6)
                r1 = gat_pool.tile([128, G * H3], dt.bfloat16, tag="hg1",
                                   bufs=6)
                for c in range(gc):
                    nc.gpsimd.indirect_dma_start(
                        out=r0[:, c * H3:(c + 1) * H3], out_offset=None,
                        in_=ztab[:],
                        in_offset=bass.IndirectOffsetOnAxis(
                            ap=h0_s[:, c0 + c:c0 + c + 1], axis=0))
                    nc.gpsimd.indirect_dma_start(
                        out=r1[:, c * H3:(c + 1) * H3], out_offset=None,
                        in_=ztab[:],
                        in_offset=bass.IndirectOffsetOnAxis(
                            ap=h1_s[:, c0 + c:c0 + c + 1], axis=0))
                if variant == 'dmaonly':
                    continue
                W = gc * H3
                e1 = work.tile([128, G * H3], dt.float32, tag="e1", bufs=4)
                nc.vector.tensor_add(out=e1[:, :W], in0=r0[:, :W],
                                     in1=r1[:, :W])
                nc.vector.tensor_add(out=e1[:, :W], in0=e1[:, :W],
                                     in1=bl1gs[:, :W])
                nc.scalar.activation(out=e1[:, :W], in_=e1[:, :W],
                                     func=mybir.ActivationFunctionType.Relu)
                nc.vector.tensor_mul(out=e1[:, :W], in0=e1[:, :W],
                                     in1=wl2gs[:, :W])
                sc = work.tile([128, G], dt.float32, tag="sc", bufs=4)
                nc.vector.reduce_sum(
                    out=sc[:, :gc],
                    in_=e1[:, :W].rearrange("p (g f) -> p g f", g=gc),
                    axis=mybir.AxisListType.X)
                nc.scalar.activation(out=out_sb[:, c0:c0 + gc],
                                     in_=sc[:, :gc],
                                     func=mybir.ActivationFunctionType.Sigmoid,
                                     bias=bl2s[:])
            nc.sync.dma_start(out=out_head[:], in_=out_sb[:])

    nc.compile()
    return nc


def _prep_inputs(x, train_edge_index, pos_edge_index, neg_edge_index,
                 W1, b1, W2, b2, Wl1, bl1, Wl2, bl2):
    """Host-side sharding / index layout. Returns (in_maps, C)."""
    x = np.asarray(x, np.float32)
    ei = np.asarray(train_edge_index)
    row, col = ei[0].astype(np.int64), ei[1].astype(np.int64)
    deg = np.bincount(col, minlength=N).astype(np.float32) + SELF_LOOP_W
    dinv = (1.0 / np.sqrt(deg)).astype(np.float32)

    import ml_dtypes
    bf16 = ml_dtypes.bfloat16
    W1 = np.asarray(W1, np.float32).astype(bf16)
    W2 = np.asarray(W2, np.float32).astype(bf16)
    Wl1 = np.asarray(Wl1, np.float32).astype(bf16)
    b1 = np.asarray(b1, np.float32)
    b2 = np.asarray(b2, np.float32)
    bl1 = np.asarray(bl1, np.float32)
    Wl2 = np.asarray(Wl2, np.float32).reshape(-1)
    bl2 = np.asarray(bl2, np.float32).reshape(-1)

    # --- per-(core,tile) edge grouping; [A|B] split by source shard-row
    # (srcr < 3200 -> half A) so the g2 AllGather can run in two pieces ---
    SRA = 3200
    core_of = col // NS
    tile_of = (col % NS) // 128
    rowp = (row // NS) * NSP + (row % NS)  # padded-global row ids
    srcr = rowp % NSP
    is_b = srcr >= SRA
    countsA = np.zeros((N_CORES, NT), np.int64)
    countsB = np.zeros((N_CORES, NT), np.int64)
    np.add.at(countsA, (core_of[~is_b], tile_of[~is_b]), 1)
    np.add.at(countsB, (core_of[is_b], tile_of[is_b]), 1)
    CtsA = [int(np.ceil(countsA[:, t].max() / 128.0)) for t in range(NT)]
    CtsB = [int(np.ceil(countsB[:, t].max() / 128.0)) for t in range(NT)]
    Cts = [a + b for a, b in zip(CtsA, CtsB)]
    bases = np.cumsum([0] + Cts)[:-1]
    CHT = int(sum(Cts))

    order = np.lexsort((tile_of, core_of))
    row_s, col_s = rowp[order], col[order]
    core_s, tile_s = core_of[order], tile_of[order]
    # boundaries per (core,tile)
    grp = core_s * NT + tile_s
    starts = np.searchsorted(grp, np.arange(N_CORES * NT))
    ends = np.searchsorted(grp, np.arange(N_CORES * NT), side='right')

    tei = np.concatenate([np.asarray(pos_edge_index),
                          np.asarray(neg_edge_index)], axis=-1)
    t0_all = tei[0].astype(np.int64)
    t1_all = tei[1].astype(np.int64)
    t0_all = (t0_all // NS) * NSP + (t0_all % NS)
    t1_all = (t1_all // NS) * NSP + (t1_all % NS)

    in_maps = []
    for k in range(N_CORES):
        lo = k * NS
        esrc = np.zeros((128, CHT), np.int32)
        esrc2 = np.zeros((128, CHT), np.int32)
        colloc = np.full((128, CHT), -1.0, np.float32)  # cast to bf16 at pack
        for t in range(NT):
            Ct, CtA = Cts[t], CtsA[t]
            base = int(bases[t])
            s, e = starts[k * NT + t], ends[k * NT + t]
            srcs = row_s[s:e]
            locs = (col_s[s:e] - lo - t * 128).astype(np.float32)
            so = np.argsort(srcs, kind='stable')
            srcs, locs = srcs[so], locs[so]
            sr = srcs % NSP
            mB = sr >= SRA
            sA, lA = srcs[~mB], locs[~mB]
            sB, lB = srcs[mB], locs[mB]
            assert len(sA) <= CtA * 128 and len(sB) <= (Ct - CtA) * 128
            full = np.zeros(Ct * 128, np.int32)
            full2 = np.zeros(Ct * 128, np.int32)
            fullc = np.full(Ct * 128, -1.0, np.float32)
            full[:len(sA)] = sA
            fullc[:len(sA)] = lA
            # half-local ids: A: 3200*k + r ; B: 3072*k + (r-3200)
            full2[:len(sA)] = (sA // NSP) * SRA + (sA % NSP)
            ob = CtA * 128
            full[ob:ob + len(sB)] = sB
            fullc[ob:ob + len(sB)] = lB
            full2[ob:ob + len(sB)] = ((sB // NSP) * (NSP - SRA)
                                      + (sB % NSP - SRA))
            esrc[:, base: base + Ct] = full.reshape(Ct, 128).T
            esrc2[:, base: base + Ct] = full2.reshape(Ct, 128).T
            colloc[:, base: base + Ct] = fullc.reshape(Ct, 128).T  # cast below

        # head edges: sort by i-row for gather locality; remember perm
        h0 = np.zeros(NHC * 128, np.int32)
        h1 = np.zeros(NHC * 128, np.int32)
        t0k = t0_all[k * HE_CORE:(k + 1) * HE_CORE]
        t1k = t1_all[k * HE_CORE:(k + 1) * HE_CORE]
        hperm = np.argsort(t0k, kind='stable')
        _HEAD_PERM[k] = hperm
        h0[:HE_CORE] = t0k[hperm]
        h1[:HE_CORE] = t1k[hperm]
        hsrc0 = h0.reshape(NHC, 128).T.copy()
        hsrc1 = h1.reshape(NHC, 128).T.copy()

        # node shard data
        xs = np.zeros((NSP, F_IN), np.float32)
        xs[:NS] = x[lo:lo + NS]
        xT = np.ascontiguousarray(xs.T).astype(bf16)
        dpm = np.zeros((128, NT), np.float32)
        dsh = np.zeros(NSP, np.float32)
        dsh[:NS] = dinv[lo:lo + NS]
        dpm[:, :] = dsh.reshape(NT, 128).T

        in_maps.append({
            "xT": xT, "dinv_pm": dpm,
            "W1": W1, "W2": W2, "Wl1": Wl1,
            "b1t": np.tile(b1[None, :], (128, 1)),
            "b2t": np.tile(b2[None, :], (128, 1)),
            "bl1t": np.tile(bl1[None, :], (128, 1)),
            "wl2bc": np.tile(Wl2[None, :], (128, 1)),
            "bl2c": np.full((128, 1), bl2[0], np.float32),
            "esrc": esrc, "esrc2": esrc2, "colloc": colloc.astype(bf16),
            "bl1g": np.tile(bl1[None, :], (128, 8)),
            "wl2g": np.tile(Wl2[None, :], (128, 8)),
            "hsrc0": hsrc0, "hsrc1": hsrc1,
        })
    return in_maps, (tuple(CtsA), tuple(CtsB))


def _get_runner(C, in_maps):
    import jax
    from concourse import bass2jax, mybir as mb
    from jax.sharding import Mesh, PartitionSpec
    from jax.experimental.shard_map import shard_map

    key = ("runner", C)
    if key in _CACHE:
        return _CACHE[key]

    nc = _CACHE.get(("nc", C))
    if nc is None:
        nc = _build_and_compile(C)
        _CACHE[("nc", C)] = nc

    bass2jax.install_neuronx_cc_hook()
    partition_name = nc.partition_id_tensor.name if nc.partition_id_tensor else None
    in_names, out_names, out_avals, zero_outs = [], [], [], []
    for a in nc.m.functions[0].allocations:
        if not isinstance(a, mb.MemoryLocationSet):
            continue
        name = a.memorylocations[0].name
        if a.kind == "ExternalInput":
            if name != partition_name:
                in_names.append(name)
        elif a.kind == "ExternalOutput":
            out_names.append(name)
            shape = tuple(a.tensor_shape)
            dtype = mb.dt.np(a.dtype)
            out_avals.append(jax.core.ShapedArray(shape, dtype))
            zero_outs.append(np.zeros(shape, dtype))
    n_params = len(in_names)
    all_in_names = in_names + out_names + ([partition_name] if partition_name else [])

    def _body(*args):
        operands = list(args)
        if partition_name is not None:
            operands.append(bass2jax.partition_id_tensor())
        outs = bass2jax._bass_exec_p.bind(
            *operands, out_avals=tuple(out_avals), in_names=tuple(all_in_names),
            out_names=tuple(out_names), lowering_input_output_aliases=(),
            sim_require_finite=True, sim_require_nnan=True, nc=nc)
        return tuple(outs)

    devices = jax.devices()[:N_CORES]
    mesh = Mesh(np.asarray(devices), ("core",))
    in_specs = (PartitionSpec("core"),) * (n_params + len(out_names))
    out_specs = (PartitionSpec("core"),) * len(out_names)
    sharded = jax.jit(shard_map(_body, mesh=mesh, in_specs=in_specs,
                                out_specs=out_specs, check_rep=False),
                      keep_unused=True)

    def run(maps):
        concat_in = [np.concatenate([np.asarray(maps[c][nm])
                                     for c in range(N_CORES)], axis=0)
                     for nm in in_names]
        concat_zero = [np.concatenate([z] * N_CORES, axis=0) for z in zero_outs]
        outs = sharded(*concat_in, *concat_zero)
        jax.block_until_ready(outs)
        return {nm: np.asarray(outs[i]) for i, nm in enumerate(out_names)}

    _CACHE[key] = run
    return run


def kernel(**inputs) -> np.ndarray:
    in_maps, C = _prep_inputs(**inputs)
    run = _get_runner(C, in_maps)
    outs = run(in_maps)
    oh = outs["out_head"].reshape(N_CORES, 128, NHC)
    res = np.empty(HEAD_E, np.float32)
    for k in range(N_CORES):
        flat = oh[k].T.reshape(-1)  # [NHC*128] in sorted-edge order
        seg = np.empty(HE_CORE, np.float32)
        seg[_HEAD_PERM[k]] = flat[:HE_CORE]
        res[k * HE_CORE:(k + 1) * HE_CORE] = seg
    return res

